# revision 1
# baseline (speedup 1.0000x reference)
"""Causal self-attention kernel for 8 Trainium2 NeuronCores.

Problem: B=2, T=2048, D=2048, H=16, Dh=128, fp32.
  qkv = x @ Wqkv + bqkv ; per-head causal attention ; out = att @ Wout + bout

Sharding (tensor parallel over heads + AllToAll before out_proj):
  Core c owns heads {2c, 2c+1}. Each core computes, for all 4096 tokens,
  Q^T/K^T (head-dim on partitions) and V (token-dim on partitions) for its
  two heads via the QKV projection with its 768-column shard of Wqkv, runs
  causal attention locally (scores are computed transposed: S^T[k,q], so
  the softmax reduction over k maps to an all-ones matmul on the partition
  axis which also broadcasts the denominator), and produces att^T
  [256, 2048] per batch. Four small AllToAlls (one per half-batch of
  tokens) redistribute from head-sharded to token-sharded; core c then
  projects its 128-token slices with the full Wout.

  Batch 0's attention is emitted interleaved with batch 1's projection so
  the PE fills the exp-latency gaps; the early AllToAlls and batch-0's
  output projection overlap batch 1's attention.

All matmuls run in float32r (full PE rate at free-dim >= 256, ~1e-4 rel
error). PSUM accumulation is fp32.
"""

import numpy as np

import concourse.bass as bass
import concourse.mybir as mybir
import concourse.tile as tile
from concourse import bacc
from concourse.bass_utils import run_bass_kernel_spmd

B, T, D, H, Dh = 2, 2048, 2048, 16, 128
NT = B * T                  # 4096 tokens total
W = 8                       # cores
HL = H // W                 # 2 heads per core
CQKV = 3 * HL * Dh          # 768 qkv columns per core
KO = D // 128               # 16 contraction subtiles
TC = 256                    # token chunk for projection rhs
NTC_B = T // TC             # 8 chunks per batch
QC = 512                    # attention q-chunk
NQC = T // QC               # 4 q-chunks per batch
HT = T // 2                 # half-batch token span (one AllToAll each)
TOKH = HT // W              # 128 tokens per core per half-batch exchange
SCALE = 1.0 / float(np.sqrt(Dh))

F32 = mybir.dt.float32
F32R = mybir.dt.float32r


def _build():
    nc = bacc.Bacc("TRN2", target_bir_lowering=False, debug=False,
                   enable_asserts=True, num_devices=W)
    xT = nc.dram_tensor("xT", [D, NT], F32, kind="ExternalInput").ap()
    wqkv = nc.dram_tensor("wqkv", [D, CQKV], F32, kind="ExternalInput").ap()
    bqkv = nc.dram_tensor("bqkv", [CQKV], F32, kind="ExternalInput").ap()
    wout = nc.dram_tensor("wout", [D, D], F32, kind="ExternalInput").ap()
    masktri = nc.dram_tensor("masktri", [128, 128], F32, kind="ExternalInput").ap()
    ones = nc.dram_tensor("ones", [128, 128], F32, kind="ExternalInput").ap()
    bvbc = nc.dram_tensor("bvbc", [128, HL * Dh], F32, kind="ExternalInput").ap()
    boutbc = nc.dram_tensor("boutbc", [128, D], F32, kind="ExternalInput").ap()
    # rows [(b*2+half)*TOKH ...): tokens [half*HT + c*TOKH ...) of batch b
    out = nc.dram_tensor("out", [B * 2 * TOKH, D], F32, kind="ExternalOutput").ap()

    xT_v = xT.rearrange("(ko p) t -> p ko t", p=128)
    wqkv_v = wqkv.rearrange("(ko p) c -> p ko c", p=128)
    wout_v = wout.rearrange("(ko p) c -> p ko c", p=128)

    with tile.TileContext(nc) as tc:
        with tc.tile_pool(name="persist", bufs=1) as persist, \
             tc.tile_pool(name="dram", bufs=1, space="DRAM") as dram_pool:
            mask_sb = persist.tile([128, 128], F32R)
            ones_sb = persist.tile([128, 128], F32R)
            bqk_sb = persist.tile([128, 2 * HL], F32)      # Q,K bias (col on partition)
            bv_sb = persist.tile([128, HL * Dh], F32)      # V bias pre-broadcast

            nc.sync.dma_start(mask_sb[:], masktri.bitcast(F32R))
            nc.sync.dma_start(ones_sb[:], ones.bitcast(F32R))
            nc.sync.dma_start(bqk_sb[:], bqkv[0:2 * HL * 128].rearrange("(cc p) -> p cc", p=128))
            nc.sync.dma_start(bv_sb[:], bvbc)

            a2a_in = [[dram_pool.tile([W, HL * 128, TOKH], F32, name=f"a2a_in{b}{h}")
                       for h in range(2)] for b in range(B)]
            a2a_out = [[dram_pool.tile([W, HL * 128, TOKH], F32, name=f"a2a_out{b}{h}")
                        for h in range(2)] for b in range(B)]

            def alloc_qkv(pool):
                qT = pool.tile([128, HL, T], F32R, name="qT")
                kT = pool.tile([128, HL, T], F32R, name="kT")
                v = pool.tile([128, HL, T // 128, Dh], F32R, name="v")
                return qT, kT, v

            def emit_proj_chunk(qkv, wqkv_sb, x_pool, proj_psum, b, tci):
                """Project one 256-token chunk of batch b into (qT, kT, v)."""
                qT_sb, kT_sb, v_sb = qkv
                t0 = b * T + tci * TC
                x_sb = x_pool.tile([128, KO, TC], F32R, name="x_sb")
                nc.sync.dma_start(x_sb[:], xT_v[:, :, t0:t0 + TC].bitcast(F32R))
                for cc in range(2 * HL):
                    ps = proj_psum.tile([128, TC], F32, name="proj_ps")
                    for ko in range(KO):
                        nc.tensor.matmul(
                            ps[:], wqkv_sb[ko][:, cc * 128:(cc + 1) * 128],
                            x_sb[:, ko, :], start=(ko == 0), stop=(ko == KO - 1))
                    dest = qT_sb if cc < HL else kT_sb
                    hl = cc if cc < HL else cc - HL
                    nc.vector.tensor_scalar_add(
                        dest[:, hl, tci * TC:(tci + 1) * TC], ps[:],
                        bqk_sb[:, cc:cc + 1])
                for tb in range(TC // 128):
                    ps = proj_psum.tile([128, HL * Dh], F32, name="proj_ps")
                    for ko in range(KO):
                        nc.tensor.matmul(
                            ps[:], x_sb[:, ko, tb * 128:(tb + 1) * 128],
                            wqkv_sb[ko][:, 2 * HL * 128:], start=(ko == 0), stop=(ko == KO - 1))
                    idx = tci * (TC // 128) + tb
                    nc.vector.tensor_tensor(
                        v_sb[:, :, idx, :],
                        ps[:].rearrange("p (hl d) -> p hl d", hl=HL),
                        bv_sb[:].rearrange("p (hl d) -> p hl d", hl=HL),
                        mybir.AluOpType.add)

            def emit_attn_group(qkv, att_sb, pools, hl, qc):
                """One (head, q-chunk) attention group: S^T -> exp -> P^T V.

                k-blocks are processed in pairs sharing one 2-bank PSUM tile
                so off-diagonal pairs need a single exp over 1024 columns.
                """
                qT_sb, kT_sb, v_sb = qkv
                ex_pool, rden_pool, s_psum, av_psum, d_psum = pools
                q0 = qc * QC
                nkb = (qc + 1) * (QC // 128)
                ps_av = av_psum.tile([128, QC], F32, name="ps_av")
                ps_dbc = d_psum.tile([128, QC], F32, name="ps_dbc")
                for kbp in range(nkb // 2):
                    kbs = (2 * kbp, 2 * kbp + 1)
                    os_ = [kb - qc * (QC // 128) for kb in kbs]
                    vss = [max(0, o) * 128 for o in os_]
                    ps_s2 = s_psum.tile([128, 2, QC], F32, name="ps_s2")
                    ex2 = ex_pool.tile([128, 2, QC], F32R, name="ex2")
                    for i, kb in enumerate(kbs):
                        nc.tensor.matmul(
                            ps_s2[:, i, vss[i]:], kT_sb[:, hl, kb * 128:(kb + 1) * 128],
                            qT_sb[:, hl, q0 + vss[i]:q0 + QC], start=True, stop=True)
                    if vss[0] == 0 and vss[1] == 0:
                        nc.scalar.activation(
                            ex2[:], ps_s2[:], mybir.ActivationFunctionType.Exp,
                            scale=SCALE)
                    else:
                        for i in range(2):
                            nc.scalar.activation(
                                ex2[:, i, vss[i]:], ps_s2[:, i, vss[i]:],
                                mybir.ActivationFunctionType.Exp, scale=SCALE)
                    for i, kb in enumerate(kbs):
                        if os_[i] >= 0:
                            nc.vector.tensor_tensor(
                                ex2[:, i, vss[i]:vss[i] + 128],
                                ex2[:, i, vss[i]:vss[i] + 128], mask_sb[:],
                                mybir.AluOpType.mult)
                        nc.tensor.matmul(
                            ps_av[:, vss[i]:], v_sb[:, hl, kb, :], ex2[:, i, vss[i]:],
                            start=(kb == 0), stop=(kb == nkb - 1))
                        nc.tensor.matmul(
                            ps_dbc[:, vss[i]:], ones_sb[:], ex2[:, i, vss[i]:],
                            start=(kb == 0), stop=(kb == nkb - 1))
                rden = rden_pool.tile([128, QC], F32, name="rden")
                nc.vector.reciprocal(rden[:], ps_dbc[:])
                nc.vector.tensor_tensor(
                    att_sb[:, hl, q0:q0 + QC], ps_av[:], rden[:],
                    mybir.AluOpType.mult)

            def emit_a2a(att_sb, b, half):
                for r in range(W):
                    nc.gpsimd.dma_start(
                        a2a_in[b][half][r].rearrange("(hl p) t -> p hl t", hl=HL, p=128),
                        att_sb[:, :, half * HT + r * TOKH:half * HT + (r + 1) * TOKH])
                nc.gpsimd.collective_compute(
                    "AllToAll", mybir.AluOpType.bypass,
                    replica_groups=[list(range(W))],
                    ins=[a2a_in[b][half][:].opt()], outs=[a2a_out[b][half][:].opt()])

            def emit_outproj(attall_pool, wout_pool, o_pool, out_psum, bout_sb, b):
                attall = []
                for half in range(2):
                    attall_sb = attall_pool.tile([128, KO, TOKH], F32R, name="attall")
                    nc.sync.dma_start(
                        attall_sb[:],
                        a2a_out[b][half][:].rearrange(
                            "r (x p) t -> p (r x) t", x=HL, p=128).bitcast(F32R))
                    attall.append(attall_sb)
                for colc in range(D // 512):
                    wout_sb = wout_pool.tile([128, KO, 512], F32R, name="wout_sb")
                    nc.sync.dma_start(
                        wout_sb[:], wout_v[:, :, colc * 512:(colc + 1) * 512].bitcast(F32R))
                    for half in (1, 0):
                        ps_o = out_psum.tile([128, 512], F32, name="ps_o")
                        for ko in range(KO):
                            nc.tensor.matmul(
                                ps_o[:], attall[half][:, ko, :],
                                wout_sb[:, ko, :], start=(ko == 0), stop=(ko == KO - 1))
                        o_sb = o_pool.tile([128, 512], F32, name="o_sb")
                        nc.vector.tensor_tensor(
                            o_sb[:], ps_o[:],
                            bout_sb[:, colc * 512:(colc + 1) * 512],
                            mybir.AluOpType.add)
                        nc.sync.dma_start(
                            out[(b * 2 + half) * TOKH:(b * 2 + half + 1) * TOKH,
                                colc * 512:(colc + 1) * 512],
                            o_sb[:])

            # heavy half (qc 2,3) first so the last A2A covers the small half
            groups_h0 = [(hl, qc) for qc in (1, 0) for hl in range(HL)]
            groups_h1 = [(hl, qc) for qc in (3, 2) for hl in range(HL)]

            with tc.tile_pool(name="qkv1_pool", bufs=1) as qkv1_pool:
                qkv1 = alloc_qkv(qkv1_pool)
                with tc.tile_pool(name="qkv0_pool", bufs=1) as qkv0_pool:
                    qkv0 = alloc_qkv(qkv0_pool)
                    with tc.tile_pool(name="att0_pool", bufs=1) as att0_pool:
                        att0_sb = att0_pool.tile([128, HL, T], F32)
                        with tc.tile_pool(name="wq_pool", bufs=1) as wq_pool, \
                             tc.tile_pool(name="x_pool", bufs=2) as x_pool, \
                             tc.tile_pool(name="proj_psum", bufs=2, space="PSUM") as proj_psum, \
                             tc.tile_pool(name="ex0_pool", bufs=2) as ex0_pool, \
                             tc.tile_pool(name="rden0_pool", bufs=1) as rden0_pool, \
                             tc.tile_pool(name="s0_psum", bufs=2, space="PSUM") as s0_psum, \
                             tc.tile_pool(name="av0_psum", bufs=1, space="PSUM") as av0_psum, \
                             tc.tile_pool(name="d0_psum", bufs=1, space="PSUM") as d0_psum:
                            wqkv_sb = [wq_pool.tile([128, CQKV], F32R,
                                                     name=f"wqkv{ko}", bufs=1)
                                       for ko in range(KO)]
                            for ko in range(KO):
                                nc.sync.dma_start(
                                    wqkv_sb[ko][:],
                                    wqkv_v[:, ko, :].bitcast(F32R))
                            pools0 = (ex0_pool, rden0_pool, s0_psum, av0_psum, d0_psum)
                            # batch-0 projection
                            for tci in range(NTC_B):
                                emit_proj_chunk(qkv0, wqkv_sb, x_pool, proj_psum, 0, tci)
                            # batch-1 projection interleaved with batch-0 attention
                            groups0 = groups_h1 + groups_h0
                            for i in range(NTC_B):
                                emit_proj_chunk(qkv1, wqkv_sb, x_pool, proj_psum, 1, i)
                                emit_attn_group(qkv0, att0_sb, pools0, *groups0[i])
                                if i == NTC_B // 2 - 1:
                                    emit_a2a(att0_sb, 0, 1)
                        emit_a2a(att0_sb, 0, 0)
                # batch-1 attention overlapping A2As and batch-0 out-proj
                with tc.tile_pool(name="att1_pool", bufs=1) as att1_pool:
                    att1_sb = att1_pool.tile([128, HL, T], F32)
                    with tc.tile_pool(name="ex1_pool", bufs=3) as ex1_pool, \
                         tc.tile_pool(name="rden1_pool", bufs=2) as rden1_pool, \
                         tc.tile_pool(name="s1_psum", bufs=2, space="PSUM") as s1_psum, \
                         tc.tile_pool(name="av1_psum", bufs=2, space="PSUM") as av1_psum, \
                         tc.tile_pool(name="d1_psum", bufs=1, space="PSUM") as d1_psum, \
                         tc.tile_pool(name="attall_pool", bufs=4) as attall_pool, \
                         tc.tile_pool(name="wout_pool", bufs=2) as wout_pool, \
                         tc.tile_pool(name="o_pool", bufs=3) as o_pool, \
                         tc.tile_pool(name="out_psum", bufs=1, space="PSUM") as out_psum:
                        bout_sb = attall_pool.tile([128, D], F32, name="bout_sb", bufs=1)
                        nc.sync.dma_start(bout_sb[:], boutbc)
                        pools1 = (ex1_pool, rden1_pool, s1_psum, av1_psum, d1_psum)
                        for g in groups_h1:
                            emit_attn_group(qkv1, att1_sb, pools1, *g)
                        emit_a2a(att1_sb, 1, 1)
                        for g in groups_h0:
                            emit_attn_group(qkv1, att1_sb, pools1, *g)
                        emit_a2a(att1_sb, 1, 0)
                        emit_outproj(attall_pool, wout_pool, o_pool, out_psum, bout_sb, 0)
                        emit_outproj(attall_pool, wout_pool, o_pool, out_psum, bout_sb, 1)
    nc.compile()
    return nc


_CACHED_NC = None


def kernel(x, Wqkv, bqkv, Wout, bout):
    global _CACHED_NC
    x = np.asarray(x, dtype=np.float32)
    Wqkv = np.asarray(Wqkv, dtype=np.float32)
    bqkv = np.asarray(bqkv, dtype=np.float32)
    Wout = np.asarray(Wout, dtype=np.float32)
    bout = np.asarray(bout, dtype=np.float32)

    if _CACHED_NC is None:
        _CACHED_NC = _build()
    nc = _CACHED_NC

    xT = np.ascontiguousarray(x.reshape(NT, D).T)          # [D, NT]
    wq4 = Wqkv.reshape(D, 3, H, Dh)                        # col = which, head, dh
    bq4 = bqkv.reshape(3, H, Dh)
    kl = np.arange(128)[:, None]
    jl = np.arange(128)[None, :]
    masktri = (jl >= kl).astype(np.float32)

    in_maps = []
    for c in range(W):
        wshard = np.ascontiguousarray(
            wq4[:, :, HL * c:HL * c + HL, :].reshape(D, CQKV))
        bshard = np.ascontiguousarray(
            bq4[:, HL * c:HL * c + HL, :].reshape(CQKV))
        in_maps.append({
            "xT": xT, "wqkv": wshard, "bqkv": bshard,
            "wout": Wout, "masktri": masktri,
            "ones": np.ones((128, 128), np.float32),
            "bvbc": np.tile(bshard[2 * HL * 128:][None, :], (128, 1)),
            "boutbc": np.tile(bout[None, :], (128, 1)),
        })

    res = run_bass_kernel_spmd(nc, in_maps, core_ids=list(range(W)))
    # res[c]["out"] rows [(b*2+h)*TOKH ...) = tokens [h*HT + c*TOKH ...) of batch b
    full = np.empty((B, T, D), np.float32)
    for c in range(W):
        for b in range(B):
            for h in range(2):
                full[b, h * HT + c * TOKH:h * HT + (c + 1) * TOKH] = \
                    res.results[c]["out"][(b * 2 + h) * TOKH:(b * 2 + h + 1) * TOKH]
    return full



# revision 2
# speedup vs baseline: 1.3172x; 1.3172x over previous
"""Causal self-attention kernel for 8 Trainium2 NeuronCores.

Problem: B=2, T=2048, D=2048, H=16, Dh=128, fp32 in/out.
  qkv = x @ Wqkv + bqkv ; per-head causal attention ; out = att @ Wout + bout

Sharding (tensor parallel over heads + AllToAll before out_proj):
  Core c owns heads {2c, 2c+1}. Each core computes, for all 4096 tokens,
  Q^T/K^T (head-dim on partitions) and V (token-dim on partitions) for its
  two heads via the QKV projection with its 768-column shard of Wqkv, runs
  causal attention locally (scores computed transposed: S^T[k,q]), and
  produces att^T [256, 2048] per batch. Four AllToAlls (one per half-batch
  of tokens) redistribute from head-sharded to token-sharded; core c then
  projects its 128-token slices with the full Wout.

v2 changes vs the fp32r baseline:
  - All matmul operands in bf16 (fp32 PSUM accumulation; ~3e-3 rel err,
    well under the 2e-2 gate). Halves HBM traffic and weight-load time.
  - Softmax denominator moved off the PE: exp blocks are summed on the
    vector engine into an fp16 accumulator; a single [128x128] ones-matmul
    per q-chunk broadcasts the partition reduction (was one ones-matmul
    per k-block = ~10% of PE work).
  - reciprocal_approx_fast instead of the slow microcoded reciprocal.
  - Wout resident in SBUF (loaded once, bf16), out-projection of both
    batches interleaved into batch-1 attention and the AllToAll windows.
  - Weight/x DMAs split across queues so the PE starts within ~5us.
"""

import numpy as np
import ml_dtypes

import concourse.bass as bass
import concourse.mybir as mybir
import concourse.tile as tile
from concourse import bacc
from concourse.bass_utils import run_bass_kernel_spmd

B, T, D, H, Dh = 2, 2048, 2048, 16, 128
NT = B * T                  # 4096 tokens total
W = 8                       # cores
HL = H // W                 # 2 heads per core
CQKV = 3 * HL * Dh          # 768 qkv columns per core
KO = D // 128               # 16 contraction subtiles
TC = 256                    # token chunk for projection rhs
NTC_B = T // TC             # 8 chunks per batch
QC = 512                    # attention q-chunk
NQC = T // QC               # 4 q-chunks per batch
HT = T // 2                 # half-batch token span (one AllToAll each)
TOKH = HT // W              # 128 tokens per core per half-batch exchange
SCALE = 1.0 / float(np.sqrt(Dh))

F32 = mybir.dt.float32
BF16 = mybir.dt.bfloat16
F16 = mybir.dt.float16


def _build():
    nc = bacc.Bacc("TRN2", target_bir_lowering=False, debug=False,
                   enable_asserts=True, num_devices=W)
    xT = nc.dram_tensor("xT", [D, NT], BF16, kind="ExternalInput").ap()
    wqkv = nc.dram_tensor("wqkv", [D, CQKV], BF16, kind="ExternalInput").ap()
    bqkv = nc.dram_tensor("bqkv", [CQKV], F32, kind="ExternalInput").ap()
    wout = nc.dram_tensor("wout", [D, D], BF16, kind="ExternalInput").ap()
    masktri = nc.dram_tensor("masktri", [128, 128], BF16, kind="ExternalInput").ap()
    ones16 = nc.dram_tensor("ones16", [128, 128], F16, kind="ExternalInput").ap()
    bvbc = nc.dram_tensor("bvbc", [128, HL * Dh], F32, kind="ExternalInput").ap()
    boutbc = nc.dram_tensor("boutbc", [128, D], F32, kind="ExternalInput").ap()
    # rows [(b*2+half)*TOKH ...): tokens [half*HT + c*TOKH ...) of batch b
    out = nc.dram_tensor("out", [B * 2 * TOKH, D], F32, kind="ExternalOutput").ap()

    xT_v = xT.rearrange("(ko p) t -> p ko t", p=128)
    wqkv_v = wqkv.rearrange("(ko p) c -> p ko c", p=128)
    wout_v = wout.rearrange("(ko p) c -> p ko c", p=128)

    with tile.TileContext(nc) as tc:
        with tc.tile_pool(name="persist", bufs=1) as persist, \
             tc.tile_pool(name="dram", bufs=1, space="DRAM") as dram_pool:
            mask_sb = persist.tile([128, 128], BF16)
            ones_sb = persist.tile([128, 128], F16)
            bqk_sb = persist.tile([128, 2 * HL], F32)      # Q,K bias (col on partition)
            bv_sb = persist.tile([128, HL * Dh], F32)      # V bias pre-broadcast
            bout_sb = persist.tile([128, D], F32)

            nc.sync.dma_start(mask_sb[:], masktri)
            nc.sync.dma_start(ones_sb[:], ones16)
            nc.sync.dma_start(bqk_sb[:], bqkv[0:2 * HL * 128].rearrange("(cc p) -> p cc", p=128))
            nc.sync.dma_start(bv_sb[:], bvbc)
            nc.sync.dma_start(bout_sb[:], boutbc)

            a2a_in = [[dram_pool.tile([W, HL * 128, TOKH], BF16, name=f"a2a_in{b}{h}")
                       for h in range(2)] for b in range(B)]
            a2a_out = [[dram_pool.tile([W, HL * 128, TOKH], BF16, name=f"a2a_out{b}{h}")
                        for h in range(2)] for b in range(B)]

            def alloc_qkv(pool):
                qT = pool.tile([128, HL, T], BF16, name="qT")
                kT = pool.tile([128, HL, T], BF16, name="kT")
                v = pool.tile([128, HL, T // 128, Dh], BF16, name="v")
                return qT, kT, v

            def emit_proj_chunk(qkv, wq_sb, x_pool, proj_psum, b, tci, dma_eng):
                """Project one 256-token chunk of batch b into (qT, kT, v)."""
                qT_sb, kT_sb, v_sb = qkv
                t0 = b * T + tci * TC
                x_sb = x_pool.tile([128, KO, TC], BF16, name="x_sb")
                dma_eng.dma_start(x_sb[:], xT_v[:, :, t0:t0 + TC])
                for cc in range(2 * HL):
                    ps = proj_psum.tile([128, TC], F32, name="proj_ps")
                    for ko in range(KO):
                        nc.tensor.matmul(
                            ps[:], wq_sb[:, ko, cc * 128:(cc + 1) * 128],
                            x_sb[:, ko, :], start=(ko == 0), stop=(ko == KO - 1))
                    dest = qT_sb if cc < HL else kT_sb
                    hl = cc if cc < HL else cc - HL
                    nc.vector.tensor_scalar_add(
                        dest[:, hl, tci * TC:(tci + 1) * TC], ps[:],
                        bqk_sb[:, cc:cc + 1])
                for tb in range(TC // 128):
                    ps = proj_psum.tile([128, HL * Dh], F32, name="proj_ps")
                    for ko in range(KO):
                        nc.tensor.matmul(
                            ps[:], x_sb[:, ko, tb * 128:(tb + 1) * 128],
                            wq_sb[:, ko, 2 * HL * 128:], start=(ko == 0), stop=(ko == KO - 1))
                    idx = tci * (TC // 128) + tb
                    nc.vector.tensor_tensor(
                        v_sb[:, :, idx, :],
                        ps[:].rearrange("p (hl d) -> p hl d", hl=HL),
                        bv_sb[:].rearrange("p (hl d) -> p hl d", hl=HL),
                        mybir.AluOpType.add)

            def emit_attn_group(qkv, att_sb, pools, hl, qc):
                """One (head, q-chunk) attention group: S^T -> exp -> P^T V.

                k-blocks are processed in pairs sharing one 2-bank PSUM tile
                so off-diagonal pairs need a single exp over 1024 columns.
                The softmax denominator is accumulated on the vector engine
                (fp16) and broadcast-reduced with one ones-matmul per group.
                """
                qT_sb, kT_sb, v_sb = qkv
                ex_pool, acc_pool, tmp_pool, rden_pool, s_psum, av_psum, den_psum = pools
                q0 = qc * QC
                nkb = (qc + 1) * (QC // 128)
                ps_av = av_psum.tile([128, QC], F32, name="ps_av")
                acc = acc_pool.tile([128, QC], F16, name="acc")
                first_acc = True
                for kbp in range(nkb // 2):
                    kbs = (2 * kbp, 2 * kbp + 1)
                    os_ = [kb - qc * (QC // 128) for kb in kbs]
                    vss = [max(0, o) * 128 for o in os_]
                    ps_s2 = s_psum.tile([128, 2, QC], F32, name="ps_s2")
                    ex2 = ex_pool.tile([128, 2, QC], BF16, name="ex2")
                    for i, kb in enumerate(kbs):
                        nc.tensor.matmul(
                            ps_s2[:, i, vss[i]:], kT_sb[:, hl, kb * 128:(kb + 1) * 128],
                            qT_sb[:, hl, q0 + vss[i]:q0 + QC], start=True, stop=True)
                    if vss[0] == 0 and vss[1] == 0:
                        nc.scalar.activation(
                            ex2[:], ps_s2[:], mybir.ActivationFunctionType.Exp,
                            scale=SCALE)
                    else:
                        for i in range(2):
                            nc.scalar.activation(
                                ex2[:, i, vss[i]:], ps_s2[:, i, vss[i]:],
                                mybir.ActivationFunctionType.Exp, scale=SCALE)
                    for i, kb in enumerate(kbs):
                        if os_[i] >= 0:
                            nc.vector.tensor_tensor(
                                ex2[:, i, vss[i]:vss[i] + 128],
                                ex2[:, i, vss[i]:vss[i] + 128], mask_sb[:],
                                mybir.AluOpType.mult)
                        nc.tensor.matmul(
                            ps_av[:, vss[i]:], v_sb[:, hl, kb, :], ex2[:, i, vss[i]:],
                            start=(kb == 0), stop=(kb == nkb - 1))
                    # denominator partial sums on DVE (fp16)
                    if vss[0] == 0 and vss[1] == 0:
                        if first_acc:
                            nc.vector.tensor_tensor(
                                acc[:], ex2[:, 0, :], ex2[:, 1, :],
                                mybir.AluOpType.add)
                            first_acc = False
                        else:
                            t = tmp_pool.tile([128, QC], F16, name="psum16")
                            nc.vector.tensor_tensor(
                                t[:], ex2[:, 0, :], ex2[:, 1, :],
                                mybir.AluOpType.add)
                            nc.vector.tensor_tensor(
                                acc[:], acc[:], t[:], mybir.AluOpType.add)
                    else:
                        if first_acc:  # qc==0 pair 0: vss == (0, 128)
                            nc.vector.tensor_copy(acc[:], ex2[:, 0, :])
                            nc.vector.tensor_tensor(
                                acc[:, vss[1]:], acc[:, vss[1]:],
                                ex2[:, 1, vss[1]:], mybir.AluOpType.add)
                            first_acc = False
                        else:
                            for i in range(2):
                                nc.vector.tensor_tensor(
                                    acc[:, vss[i]:], acc[:, vss[i]:],
                                    ex2[:, i, vss[i]:], mybir.AluOpType.add)
                ps_den = den_psum.tile([128, QC], F32, name="ps_den")
                nc.tensor.matmul(ps_den[:], ones_sb[:], acc[:], start=True, stop=True)
                rden = rden_pool.tile([128, QC], F32, name="rden")
                nc.vector.reciprocal_approx_fast(rden[:], ps_den[:])
                nc.vector.tensor_tensor(
                    att_sb[:, hl, q0:q0 + QC], ps_av[:], rden[:],
                    mybir.AluOpType.mult)

            def emit_a2a(att_sb, b, half):
                for r in range(W):
                    nc.gpsimd.dma_start(
                        a2a_in[b][half][r].rearrange("(hl p) t -> p hl t", hl=HL, p=128),
                        att_sb[:, :, half * HT + r * TOKH:half * HT + (r + 1) * TOKH])
                nc.gpsimd.collective_compute(
                    "AllToAll", mybir.AluOpType.bypass,
                    replica_groups=[list(range(W))],
                    ins=[a2a_in[b][half][:].opt()], outs=[a2a_out[b][half][:].opt()])

            # heavy half (qc 3,2) first so the last A2A covers the small half
            groups_h1 = [(hl, qc) for qc in (3, 2) for hl in range(HL)]
            groups_h0 = [(hl, qc) for qc in (1, 0) for hl in range(HL)]

            with tc.tile_pool(name="qkv1_pool", bufs=1) as qkv1_pool:
                qkv1 = alloc_qkv(qkv1_pool)
                with tc.tile_pool(name="qkv0_pool", bufs=1) as qkv0_pool:
                    qkv0 = alloc_qkv(qkv0_pool)
                    with tc.tile_pool(name="att0_pool", bufs=1) as att0_pool:
                        att0_sb = att0_pool.tile([128, HL, T], BF16)
                        with tc.tile_pool(name="wq_pool", bufs=1) as wq_pool, \
                             tc.tile_pool(name="x_pool", bufs=2) as x_pool, \
                             tc.tile_pool(name="proj_psum", bufs=2, space="PSUM") as proj_psum, \
                             tc.tile_pool(name="ex0_pool", bufs=3) as ex0_pool, \
                             tc.tile_pool(name="acc0_pool", bufs=2) as acc0_pool, \
                             tc.tile_pool(name="tmp0_pool", bufs=2) as tmp0_pool, \
                             tc.tile_pool(name="rden0_pool", bufs=2) as rden0_pool, \
                             tc.tile_pool(name="s0_psum", bufs=2, space="PSUM") as s0_psum, \
                             tc.tile_pool(name="av0_psum", bufs=1, space="PSUM") as av0_psum, \
                             tc.tile_pool(name="d0_psum", bufs=1, space="PSUM") as d0_psum:
                            wq_sb = wq_pool.tile([128, KO, CQKV], BF16, name="wq_sb")
                            for p4 in range(0, KO, 4):
                                nc.sync.dma_start(
                                    wq_sb[:, p4:p4 + 4, :], wqkv_v[:, p4:p4 + 4, :])
                            pools0 = (ex0_pool, acc0_pool, tmp0_pool, rden0_pool,
                                      s0_psum, av0_psum, d0_psum)
                            # batch-0 projection (first x chunks race on the
                            # scalar DMA queue so the PE starts early)
                            for tci in range(NTC_B):
                                emit_proj_chunk(qkv0, wq_sb, x_pool, proj_psum, 0, tci,
                                                nc.scalar if tci < 2 else nc.sync)
                            # batch-1 projection interleaved with batch-0 attention
                            groups0 = groups_h1 + groups_h0
                            for i in range(NTC_B):
                                emit_proj_chunk(qkv1, wq_sb, x_pool, proj_psum, 1, i,
                                                nc.sync)
                                emit_attn_group(qkv0, att0_sb, pools0, *groups0[i])
                                if i == NTC_B // 2 - 1:
                                    emit_a2a(att0_sb, 0, 1)
                        emit_a2a(att0_sb, 0, 0)
                # batch-1 attention overlapping A2As and both out-projections
                with tc.tile_pool(name="att1_pool", bufs=1) as att1_pool:
                    att1_sb = att1_pool.tile([128, HL, T], BF16)
                    with tc.tile_pool(name="wout_pool", bufs=1) as wout_pool, \
                         tc.tile_pool(name="attall_pool", bufs=4) as attall_pool, \
                         tc.tile_pool(name="o_pool", bufs=3) as o_pool, \
                         tc.tile_pool(name="out_psum", bufs=2, space="PSUM") as out_psum, \
                         tc.tile_pool(name="ex1_pool", bufs=3) as ex1_pool, \
                         tc.tile_pool(name="acc1_pool", bufs=2) as acc1_pool, \
                         tc.tile_pool(name="tmp1_pool", bufs=2) as tmp1_pool, \
                         tc.tile_pool(name="rden1_pool", bufs=2) as rden1_pool, \
                         tc.tile_pool(name="s1_psum", bufs=2, space="PSUM") as s1_psum, \
                         tc.tile_pool(name="av1_psum", bufs=1, space="PSUM") as av1_psum, \
                         tc.tile_pool(name="d1_psum", bufs=1, space="PSUM") as d1_psum:
                        wout_sb = wout_pool.tile([128, KO, D], BF16, name="wout_sb")
                        for colc in range(4):
                            nc.sync.dma_start(
                                wout_sb[:, :, colc * 512:(colc + 1) * 512],
                                wout_v[:, :, colc * 512:(colc + 1) * 512])
                        pools1 = (ex1_pool, acc1_pool, tmp1_pool, rden1_pool,
                                  s1_psum, av1_psum, d1_psum)

                        attall = {}

                        def load_attall(b, half):
                            t = attall_pool.tile([128, KO, TOKH], BF16, name="attall")
                            nc.sync.dma_start(
                                t[:],
                                a2a_out[b][half][:].rearrange(
                                    "r (x p) t -> p (r x) t", x=HL, p=128))
                            attall[(b, half)] = t

                        def emit_outproj_group(b, half, colc):
                            ps_o = out_psum.tile([128, 512], F32, name="ps_o")
                            for ko in range(KO):
                                nc.tensor.matmul(
                                    ps_o[:], attall[(b, half)][:, ko, :],
                                    wout_sb[:, ko, colc * 512:(colc + 1) * 512],
                                    start=(ko == 0), stop=(ko == KO - 1))
                            o_sb = o_pool.tile([128, 512], F32, name="o_sb")
                            nc.vector.tensor_tensor(
                                o_sb[:], ps_o[:],
                                bout_sb[:, colc * 512:(colc + 1) * 512],
                                mybir.AluOpType.add)
                            nc.sync.dma_start(
                                out[(b * 2 + half) * TOKH:(b * 2 + half + 1) * TOKH,
                                    colc * 512:(colc + 1) * 512],
                                o_sb[:])

                        load_attall(0, 1)
                        load_attall(0, 0)
                        # phase A: heavy-half b1 attention + first b0 outproj
                        emit_attn_group(qkv1, att1_sb, pools1, *groups_h1[0])
                        emit_outproj_group(0, 1, 0)
                        emit_outproj_group(0, 0, 0)
                        emit_attn_group(qkv1, att1_sb, pools1, *groups_h1[1])
                        emit_outproj_group(0, 1, 1)
                        emit_outproj_group(0, 0, 1)
                        emit_attn_group(qkv1, att1_sb, pools1, *groups_h1[2])
                        emit_attn_group(qkv1, att1_sb, pools1, *groups_h1[3])
                        emit_a2a(att1_sb, 1, 1)
                        load_attall(1, 1)
                        # phase B: light-half b1 attention + remaining b0 outproj
                        emit_attn_group(qkv1, att1_sb, pools1, *groups_h0[0])
                        emit_outproj_group(0, 1, 2)
                        emit_outproj_group(0, 0, 2)
                        emit_attn_group(qkv1, att1_sb, pools1, *groups_h0[1])
                        emit_outproj_group(0, 1, 3)
                        emit_outproj_group(0, 0, 3)
                        emit_attn_group(qkv1, att1_sb, pools1, *groups_h0[2])
                        emit_attn_group(qkv1, att1_sb, pools1, *groups_h0[3])
                        emit_a2a(att1_sb, 1, 0)
                        # phase C: b1 outproj (h1's A2A landed during phase B)
                        for colc in range(4):
                            emit_outproj_group(1, 1, colc)
                        load_attall(1, 0)
                        for colc in range(4):
                            emit_outproj_group(1, 0, colc)
    nc.compile()
    return nc


_CACHED_NC = None


def kernel(x, Wqkv, bqkv, Wout, bout):
    global _CACHED_NC
    x = np.asarray(x, dtype=np.float32)
    Wqkv = np.asarray(Wqkv, dtype=np.float32)
    bqkv = np.asarray(bqkv, dtype=np.float32)
    Wout = np.asarray(Wout, dtype=np.float32)
    bout = np.asarray(bout, dtype=np.float32)

    if _CACHED_NC is None:
        _CACHED_NC = _build()
    nc = _CACHED_NC

    bf16 = ml_dtypes.bfloat16
    xT = np.ascontiguousarray(x.reshape(NT, D).T).astype(bf16)   # [D, NT]
    wq4 = Wqkv.reshape(D, 3, H, Dh)                              # col = which, head, dh
    bq4 = bqkv.reshape(3, H, Dh)
    wout16 = Wout.astype(bf16)
    kl = np.arange(128)[:, None]
    jl = np.arange(128)[None, :]
    masktri = (jl >= kl).astype(bf16)

    in_maps = []
    for c in range(W):
        wshard = np.ascontiguousarray(
            wq4[:, :, HL * c:HL * c + HL, :].reshape(D, CQKV)).astype(bf16)
        bshard = np.ascontiguousarray(
            bq4[:, HL * c:HL * c + HL, :].reshape(CQKV))
        in_maps.append({
            "xT": xT, "wqkv": wshard, "bqkv": bshard,
            "wout": wout16, "masktri": masktri,
            "ones16": np.ones((128, 128), np.float16),
            "bvbc": np.tile(bshard[2 * HL * 128:][None, :], (128, 1)),
            "boutbc": np.tile(bout[None, :], (128, 1)),
        })

    res = run_bass_kernel_spmd(nc, in_maps, core_ids=list(range(W)))
    # res[c]["out"] rows [(b*2+h)*TOKH ...) = tokens [h*HT + c*TOKH ...) of batch b
    full = np.empty((B, T, D), np.float32)
    for c in range(W):
        for b in range(B):
            for h in range(2):
                full[b, h * HT + c * TOKH:h * HT + (c + 1) * TOKH] = \
                    res.results[c]["out"][(b * 2 + h) * TOKH:(b * 2 + h + 1) * TOKH]
    return full


# revision 3
# speedup vs baseline: 1.3975x; 1.0609x over previous
"""Causal self-attention kernel for 8 Trainium2 NeuronCores.

Problem: B=2, T=2048, D=2048, H=16, Dh=128, fp32 in/out.
  qkv = x @ Wqkv + bqkv ; per-head causal attention ; out = att @ Wout + bout

Sharding (tensor parallel over heads + AllToAll before out_proj):
  Core c owns heads {2c, 2c+1}. Each core computes, for all 4096 tokens,
  Q^T/K^T (head-dim on partitions) and V (token-dim on partitions) for its
  two heads via the QKV projection with its 768-column shard of Wqkv, runs
  causal attention locally (scores computed transposed: S^T[k,q]), and
  produces att^T [256, 2048] per batch. Four AllToAlls (one per half-batch
  of tokens) redistribute from head-sharded to token-sharded; core c then
  projects its 128-token slices with the full Wout.

All matmul operands are bf16 (fp32 PSUM accumulation, ~4e-3 rel err).
The softmax denominator is accumulated on the vector engine (fp16) with
one [128x128] ones-matmul per q-chunk instead of one per k-block.
Host-side pre-swizzled DRAM layouts give contiguous >=1KB DMA lines.
The schedule drains attention as early as the projection allows: q-chunks
0/1 of each batch run inside that batch's projection window, so three of
the four AllToAlls fire before the tail; the tail is batch-1's heavy-half
attention with both batches' output projection as PE filler, and only the
last AllToAll (plus its 15us projection) is exposed.
"""

import numpy as np
import ml_dtypes

import concourse.bass as bass
import concourse.mybir as mybir
import concourse.tile as tile
from concourse import bacc
from concourse.bass_utils import run_bass_kernel_spmd

B, T, D, H, Dh = 2, 2048, 2048, 16, 128
NT = B * T                  # 4096 tokens total
W = 8                       # cores
HL = H // W                 # 2 heads per core
CQKV = 3 * HL * Dh          # 768 qkv columns per core
NCC = CQKV // 128           # 6 column groups of the qkv weight shard
KO = D // 128               # 16 contraction subtiles
QC = 512                    # attention q-chunk
NQC = T // QC               # 4 q-chunks per batch
HT = T // 2                 # half-batch token span (one AllToAll each)
TOKH = HT // W              # 128 tokens per core per half-batch exchange
SCALE = 1.0 / float(np.sqrt(Dh))

# projection token chunks per batch: two 256s first (small startup critical
# path + feeds attention q-chunk 0 early), then three 512s
CHUNKS = [(0, 256), (256, 256), (512, 512), (1024, 512), (1536, 512)]

F32 = mybir.dt.float32
BF16 = mybir.dt.bfloat16
F16 = mybir.dt.float16


def _build():
    nc = bacc.Bacc("TRN2", target_bir_lowering=False, debug=False,
                   enable_asserts=True, num_devices=W)
    # pre-swizzled layouts (see kernel() for the host-side transposes)
    xp = nc.dram_tensor("xp", [128, KO, NT], BF16, kind="ExternalInput").ap()
    wqkv = nc.dram_tensor("wqkv", [128, NCC, KO, 128], BF16, kind="ExternalInput").ap()
    bqkv = nc.dram_tensor("bqkv", [CQKV], F32, kind="ExternalInput").ap()
    wout = nc.dram_tensor("wout", [128, 4, KO, 512], BF16, kind="ExternalInput").ap()
    masktri = nc.dram_tensor("masktri", [128, 128], BF16, kind="ExternalInput").ap()
    ones16 = nc.dram_tensor("ones16", [128, 128], F16, kind="ExternalInput").ap()
    bvbc = nc.dram_tensor("bvbc", [128, HL * Dh], F32, kind="ExternalInput").ap()
    boutbc = nc.dram_tensor("boutbc", [128, D], F32, kind="ExternalInput").ap()
    # rows [(b*2+half)*TOKH ...): tokens [half*HT + c*TOKH ...) of batch b
    out = nc.dram_tensor("out", [B * 2 * TOKH, D], F32, kind="ExternalOutput").ap()

    with tile.TileContext(nc) as tc:
        with tc.tile_pool(name="persist", bufs=1) as persist, \
             tc.tile_pool(name="dram", bufs=1, space="DRAM") as dram_pool:
            mask_sb = persist.tile([128, 128], BF16)
            ones_sb = persist.tile([128, 128], F16)
            bqk_sb = persist.tile([128, 2 * HL], F32)      # Q,K bias (col on partition)
            bv_sb = persist.tile([128, HL * Dh], F32)      # V bias pre-broadcast
            bout_sb = persist.tile([128, D], F32)

            nc.sync.dma_start(mask_sb[:], masktri)
            nc.sync.dma_start(ones_sb[:], ones16)
            nc.sync.dma_start(bqk_sb[:], bqkv[0:2 * HL * 128].rearrange("(cc p) -> p cc", p=128))
            nc.sync.dma_start(bv_sb[:], bvbc)
            nc.sync.dma_start(bout_sb[:], boutbc)

            a2a_in = [[dram_pool.tile([W, HL * 128, TOKH], BF16, name=f"a2a_in{b}{h}")
                       for h in range(2)] for b in range(B)]
            a2a_out = [[dram_pool.tile([W, HL * 128, TOKH], BF16, name=f"a2a_out{b}{h}")
                        for h in range(2)] for b in range(B)]

            def alloc_qkv(pool):
                qT = pool.tile([128, HL, T], BF16, name="qT")
                kT = pool.tile([128, HL, T], BF16, name="kT")
                v = pool.tile([128, HL, T // 128, Dh], BF16, name="v")
                return qT, kT, v

            def emit_proj_chunk(qkv, wq_sb, x_pool, proj_psum, b, chunk, dma_eng):
                """Project one token chunk of batch b into (qT, kT, v)."""
                qT_sb, kT_sb, v_sb = qkv
                toff, tc_sz = chunk
                t0 = b * T + toff
                x_sb = x_pool.tile([128, KO, tc_sz], BF16, name="x_sb")
                dma_eng.dma_start(x_sb[:], xp[:, :, t0:t0 + tc_sz])
                for cc in range(2 * HL):
                    ps = proj_psum.tile([128, tc_sz], F32, name="proj_ps")
                    for ko in range(KO):
                        nc.tensor.matmul(
                            ps[:], wq_sb[:, cc, ko, :],
                            x_sb[:, ko, :], start=(ko == 0), stop=(ko == KO - 1))
                    dest = qT_sb if cc < HL else kT_sb
                    hl = cc if cc < HL else cc - HL
                    nc.vector.tensor_scalar_add(
                        dest[:, hl, toff:toff + tc_sz], ps[:],
                        bqk_sb[:, cc:cc + 1])
                for tb in range(tc_sz // 128):
                    ps = proj_psum.tile([128, HL * Dh], F32, name="proj_ps")
                    for ko in range(KO):
                        nc.tensor.matmul(
                            ps[:], x_sb[:, ko, tb * 128:(tb + 1) * 128],
                            wq_sb[:, 2 * HL:, ko, :], start=(ko == 0), stop=(ko == KO - 1))
                    idx = toff // 128 + tb
                    nc.vector.tensor_tensor(
                        v_sb[:, :, idx, :],
                        ps[:].rearrange("p (hl d) -> p hl d", hl=HL),
                        bv_sb[:].rearrange("p (hl d) -> p hl d", hl=HL),
                        mybir.AluOpType.add)

            def emit_attn_group(qkv, att_sb, pools, hl, qc):
                """One (head, q-chunk) attention group: S^T -> exp -> P^T V.

                k-blocks are processed in pairs sharing one 2-bank PSUM tile
                so off-diagonal pairs need a single exp over 1024 columns.
                The softmax denominator is accumulated on the vector engine
                (fp16) and broadcast-reduced with one ones-matmul per group.
                """
                qT_sb, kT_sb, v_sb = qkv
                ex_pool, acc_pool, tmp_pool, rden_pool, s_psum, av_psum, den_psum = pools
                q0 = qc * QC
                nkb = (qc + 1) * (QC // 128)
                ps_av = av_psum.tile([128, QC], F32, name="ps_av")
                acc = acc_pool.tile([128, QC], F16, name="acc")
                first_acc = True
                for kbp in range(nkb // 2):
                    kbs = (2 * kbp, 2 * kbp + 1)
                    os_ = [kb - qc * (QC // 128) for kb in kbs]
                    vss = [max(0, o) * 128 for o in os_]
                    ps_s2 = s_psum.tile([128, 2, QC], F32, name="ps_s2")
                    ex2 = ex_pool.tile([128, 2, QC], BF16, name="ex2")
                    for i, kb in enumerate(kbs):
                        nc.tensor.matmul(
                            ps_s2[:, i, vss[i]:], kT_sb[:, hl, kb * 128:(kb + 1) * 128],
                            qT_sb[:, hl, q0 + vss[i]:q0 + QC], start=True, stop=True)
                    if vss[0] == 0 and vss[1] == 0:
                        nc.scalar.activation(
                            ex2[:], ps_s2[:], mybir.ActivationFunctionType.Exp,
                            scale=SCALE)
                    else:
                        for i in range(2):
                            nc.scalar.activation(
                                ex2[:, i, vss[i]:], ps_s2[:, i, vss[i]:],
                                mybir.ActivationFunctionType.Exp, scale=SCALE)
                    for i, kb in enumerate(kbs):
                        if os_[i] >= 0:
                            nc.vector.tensor_tensor(
                                ex2[:, i, vss[i]:vss[i] + 128],
                                ex2[:, i, vss[i]:vss[i] + 128], mask_sb[:],
                                mybir.AluOpType.mult)
                        nc.tensor.matmul(
                            ps_av[:, vss[i]:], v_sb[:, hl, kb, :], ex2[:, i, vss[i]:],
                            start=(kb == 0), stop=(kb == nkb - 1))
                    # denominator partial sums on DVE (fp16)
                    if vss[0] == 0 and vss[1] == 0:
                        if first_acc:
                            nc.vector.tensor_tensor(
                                acc[:], ex2[:, 0, :], ex2[:, 1, :],
                                mybir.AluOpType.add)
                            first_acc = False
                        else:
                            t = tmp_pool.tile([128, QC], F16, name="psum16")
                            nc.vector.tensor_tensor(
                                t[:], ex2[:, 0, :], ex2[:, 1, :],
                                mybir.AluOpType.add)
                            nc.vector.tensor_tensor(
                                acc[:], acc[:], t[:], mybir.AluOpType.add)
                    else:
                        if first_acc:  # qc==0 pair 0: vss == (0, 128)
                            nc.vector.tensor_copy(acc[:], ex2[:, 0, :])
                            nc.vector.tensor_tensor(
                                acc[:, vss[1]:], acc[:, vss[1]:],
                                ex2[:, 1, vss[1]:], mybir.AluOpType.add)
                            first_acc = False
                        else:
                            for i in range(2):
                                nc.vector.tensor_tensor(
                                    acc[:, vss[i]:], acc[:, vss[i]:],
                                    ex2[:, i, vss[i]:], mybir.AluOpType.add)
                ps_den = den_psum.tile([128, QC], F32, name="ps_den")
                nc.tensor.matmul(ps_den[:], ones_sb[:], acc[:], start=True, stop=True)
                rden = rden_pool.tile([128, QC], F32, name="rden")
                nc.vector.reciprocal_approx_fast(rden[:], ps_den[:])
                nc.vector.tensor_tensor(
                    att_sb[:, hl, q0:q0 + QC], ps_av[:], rden[:],
                    mybir.AluOpType.mult)

            def emit_a2a(att_sb, b, half):
                for r in range(W):
                    nc.gpsimd.dma_start(
                        a2a_in[b][half][r].rearrange("(hl p) t -> p hl t", hl=HL, p=128),
                        att_sb[:, :, half * HT + r * TOKH:half * HT + (r + 1) * TOKH])
                nc.gpsimd.collective_compute(
                    "AllToAll", mybir.AluOpType.bypass,
                    replica_groups=[list(range(W))],
                    ins=[a2a_in[b][half][:].opt()], outs=[a2a_out[b][half][:].opt()])

            with tc.tile_pool(name="qkv1_pool", bufs=1) as qkv1_pool:
                qkv1 = alloc_qkv(qkv1_pool)
                with tc.tile_pool(name="att1_pool", bufs=1) as att1_pool:
                    att1_sb = att1_pool.tile([128, HL, T], BF16)
                    with tc.tile_pool(name="qkv0_pool", bufs=1) as qkv0_pool:
                        qkv0 = alloc_qkv(qkv0_pool)
                        with tc.tile_pool(name="att0_pool", bufs=1) as att0_pool:
                            att0_sb = att0_pool.tile([128, HL, T], BF16)
                            with tc.tile_pool(name="wq_pool", bufs=1) as wq_pool, \
                                 tc.tile_pool(name="x_pool", bufs=3) as x_pool, \
                                 tc.tile_pool(name="proj_psum", bufs=2, space="PSUM") as proj_psum, \
                                 tc.tile_pool(name="ex0_pool", bufs=3) as ex0_pool, \
                                 tc.tile_pool(name="acc0_pool", bufs=2) as acc0_pool, \
                                 tc.tile_pool(name="tmp0_pool", bufs=2) as tmp0_pool, \
                                 tc.tile_pool(name="rden0_pool", bufs=2) as rden0_pool, \
                                 tc.tile_pool(name="s0_psum", bufs=2, space="PSUM") as s0_psum, \
                                 tc.tile_pool(name="av0_psum", bufs=1, space="PSUM") as av0_psum, \
                                 tc.tile_pool(name="d0_psum", bufs=1, space="PSUM") as d0_psum:
                                wq_sb = wq_pool.tile([128, NCC, KO, 128], BF16, name="wq_sb")
                                # split by column group: psum group cc starts
                                # after only its 0.5MB weight slice arrives
                                for cc in range(NCC):
                                    nc.sync.dma_start(wq_sb[:, cc], wqkv[:, cc])
                                pools0 = (ex0_pool, acc0_pool, tmp0_pool, rden0_pool,
                                          s0_psum, av0_psum, d0_psum)
                                # ---- phase P0: batch-0 projection + its early
                                # attention q-chunks (first x chunks race on the
                                # scalar DMA queue so the PE starts early)
                                emit_proj_chunk(qkv0, wq_sb, x_pool, proj_psum, 0, CHUNKS[0], nc.scalar)
                                emit_proj_chunk(qkv0, wq_sb, x_pool, proj_psum, 0, CHUNKS[1], nc.scalar)
                                emit_attn_group(qkv0, att0_sb, pools0, 0, 0)
                                emit_attn_group(qkv0, att0_sb, pools0, 1, 0)
                                emit_proj_chunk(qkv0, wq_sb, x_pool, proj_psum, 0, CHUNKS[2], nc.sync)
                                emit_attn_group(qkv0, att0_sb, pools0, 0, 1)
                                emit_attn_group(qkv0, att0_sb, pools0, 1, 1)
                                emit_proj_chunk(qkv0, wq_sb, x_pool, proj_psum, 0, CHUNKS[3], nc.sync)
                                emit_proj_chunk(qkv0, wq_sb, x_pool, proj_psum, 0, CHUNKS[4], nc.sync)
                                emit_a2a(att0_sb, 0, 0)
                                # ---- phase P1: batch-1 projection + b0 heavy-half
                                # attention + b1 light-half attention
                                emit_proj_chunk(qkv1, wq_sb, x_pool, proj_psum, 1, CHUNKS[0], nc.sync)
                                emit_proj_chunk(qkv1, wq_sb, x_pool, proj_psum, 1, CHUNKS[1], nc.sync)
                                emit_attn_group(qkv0, att0_sb, pools0, 0, 3)
                                emit_attn_group(qkv0, att0_sb, pools0, 1, 3)
                                emit_proj_chunk(qkv1, wq_sb, x_pool, proj_psum, 1, CHUNKS[2], nc.sync)
                                emit_attn_group(qkv0, att0_sb, pools0, 0, 2)
                                emit_attn_group(qkv0, att0_sb, pools0, 1, 2)
                                emit_a2a(att0_sb, 0, 1)
                                emit_attn_group(qkv1, att1_sb, pools0, 0, 0)
                                emit_attn_group(qkv1, att1_sb, pools0, 1, 0)
                                emit_proj_chunk(qkv1, wq_sb, x_pool, proj_psum, 1, CHUNKS[3], nc.sync)
                                emit_attn_group(qkv1, att1_sb, pools0, 0, 1)
                                emit_attn_group(qkv1, att1_sb, pools0, 1, 1)
                                emit_proj_chunk(qkv1, wq_sb, x_pool, proj_psum, 1, CHUNKS[4], nc.sync)
                                emit_a2a(att1_sb, 1, 0)
                        # qkv0/att0 freed; tail: b1 heavy-half attention with
                        # both batches' output projection as PE filler
                    with tc.tile_pool(name="wout_pool", bufs=1) as wout_pool, \
                         tc.tile_pool(name="attall_pool", bufs=4) as attall_pool, \
                         tc.tile_pool(name="o_pool", bufs=3) as o_pool, \
                         tc.tile_pool(name="out_psum", bufs=2, space="PSUM") as out_psum, \
                         tc.tile_pool(name="ex1_pool", bufs=3) as ex1_pool, \
                         tc.tile_pool(name="acc1_pool", bufs=2) as acc1_pool, \
                         tc.tile_pool(name="tmp1_pool", bufs=2) as tmp1_pool, \
                         tc.tile_pool(name="rden1_pool", bufs=2) as rden1_pool, \
                         tc.tile_pool(name="s1_psum", bufs=2, space="PSUM") as s1_psum, \
                         tc.tile_pool(name="av1_psum", bufs=1, space="PSUM") as av1_psum, \
                         tc.tile_pool(name="d1_psum", bufs=1, space="PSUM") as d1_psum:
                        wout_sb = wout_pool.tile([128, 4, KO, 512], BF16, name="wout_sb")
                        for colc in range(4):
                            nc.sync.dma_start(wout_sb[:, colc], wout[:, colc])
                        pools1 = (ex1_pool, acc1_pool, tmp1_pool, rden1_pool,
                                  s1_psum, av1_psum, d1_psum)

                        attall = {}

                        def load_attall(b, half):
                            t = attall_pool.tile([128, KO, TOKH], BF16, name="attall")
                            nc.sync.dma_start(
                                t[:],
                                a2a_out[b][half][:].rearrange(
                                    "r (x p) t -> p (r x) t", x=HL, p=128))
                            attall[(b, half)] = t

                        def emit_outproj_group(b, half, colc):
                            ps_o = out_psum.tile([128, 512], F32, name="ps_o")
                            for ko in range(KO):
                                nc.tensor.matmul(
                                    ps_o[:], attall[(b, half)][:, ko, :],
                                    wout_sb[:, colc, ko, :],
                                    start=(ko == 0), stop=(ko == KO - 1))
                            o_sb = o_pool.tile([128, 512], F32, name="o_sb")
                            nc.vector.tensor_tensor(
                                o_sb[:], ps_o[:],
                                bout_sb[:, colc * 512:(colc + 1) * 512],
                                mybir.AluOpType.add)
                            nc.sync.dma_start(
                                out[(b * 2 + half) * TOKH:(b * 2 + half + 1) * TOKH,
                                    colc * 512:(colc + 1) * 512],
                                o_sb[:])

                        load_attall(0, 0)   # A2A fired at end of P0
                        load_attall(0, 1)   # fired mid-P1
                        load_attall(1, 0)   # fired at end of P1
                        # b1 heavy-half attention, outproj groups fill exp gaps
                        emit_attn_group(qkv1, att1_sb, pools1, 0, 3)
                        emit_outproj_group(0, 0, 0)
                        emit_outproj_group(0, 0, 1)
                        emit_attn_group(qkv1, att1_sb, pools1, 1, 3)
                        emit_outproj_group(0, 0, 2)
                        emit_outproj_group(0, 0, 3)
                        emit_attn_group(qkv1, att1_sb, pools1, 0, 2)
                        emit_outproj_group(0, 1, 0)
                        emit_outproj_group(0, 1, 1)
                        emit_attn_group(qkv1, att1_sb, pools1, 1, 2)
                        emit_a2a(att1_sb, 1, 1)
                        load_attall(1, 1)
                        emit_outproj_group(0, 1, 2)
                        emit_outproj_group(0, 1, 3)
                        for colc in range(4):
                            emit_outproj_group(1, 0, colc)
                        for colc in range(4):
                            emit_outproj_group(1, 1, colc)
    nc.compile()
    return nc


_CACHED_NC = None


def kernel(x, Wqkv, bqkv, Wout, bout):
    global _CACHED_NC
    x = np.asarray(x, dtype=np.float32)
    Wqkv = np.asarray(Wqkv, dtype=np.float32)
    bqkv = np.asarray(bqkv, dtype=np.float32)
    Wout = np.asarray(Wout, dtype=np.float32)
    bout = np.asarray(bout, dtype=np.float32)

    if _CACHED_NC is None:
        _CACHED_NC = _build()
    nc = _CACHED_NC

    bf16 = ml_dtypes.bfloat16
    # xp[p, ko, t] = x[t, ko*128+p]
    xp = np.ascontiguousarray(
        x.reshape(NT, KO, 128).transpose(2, 1, 0)).astype(bf16)
    wq4 = Wqkv.reshape(D, 3, H, Dh)                              # col = which, head, dh
    bq4 = bqkv.reshape(3, H, Dh)
    # wout_pre[p, colc, ko, c] = Wout[ko*128+p, colc*512+c]
    wout_pre = np.ascontiguousarray(
        Wout.reshape(KO, 128, 4, 512).transpose(1, 2, 0, 3)).astype(bf16)
    kl = np.arange(128)[:, None]
    jl = np.arange(128)[None, :]
    masktri = (jl >= kl).astype(bf16)

    in_maps = []
    for c in range(W):
        wshard = np.ascontiguousarray(
            wq4[:, :, HL * c:HL * c + HL, :].reshape(D, CQKV))
        # wpre[p, cc, ko, ccol] = wshard[ko*128+p, cc*128+ccol]
        wpre = np.ascontiguousarray(
            wshard.reshape(KO, 128, NCC, 128).transpose(1, 2, 0, 3)).astype(bf16)
        bshard = np.ascontiguousarray(
            bq4[:, HL * c:HL * c + HL, :].reshape(CQKV))
        in_maps.append({
            "xp": xp, "wqkv": wpre, "bqkv": bshard,
            "wout": wout_pre, "masktri": masktri,
            "ones16": np.ones((128, 128), np.float16),
            "bvbc": np.tile(bshard[2 * HL * 128:][None, :], (128, 1)),
            "boutbc": np.tile(bout[None, :], (128, 1)),
        })

    res = run_bass_kernel_spmd(nc, in_maps, core_ids=list(range(W)))
    # res[c]["out"] rows [(b*2+h)*TOKH ...) = tokens [h*HT + c*TOKH ...) of batch b
    full = np.empty((B, T, D), np.float32)
    for c in range(W):
        for b in range(B):
            for h in range(2):
                full[b, h * HT + c * TOKH:h * HT + (c + 1) * TOKH] = \
                    res.results[c]["out"][(b * 2 + h) * TOKH:(b * 2 + h + 1) * TOKH]
    return full


# revision 8
# speedup vs baseline: 1.4624x; 1.0465x over previous
"""Causal self-attention kernel for 8 Trainium2 NeuronCores.

Problem: B=2, T=2048, D=2048, H=16, Dh=128, fp32 in/out.
  qkv = x @ Wqkv + bqkv ; per-head causal attention ; out = att @ Wout + bout

Sharding (tensor parallel over heads + AllToAll before out_proj):
  Core c owns heads {2c, 2c+1}. Each core computes, for all 4096 tokens,
  Q^T/K^T (head-dim on partitions) and V (token-dim on partitions) for its
  two heads via the QKV projection with its 768-column shard of Wqkv, runs
  causal attention locally (scores computed transposed: S^T[k,q]), and
  produces att^T [256, 2048] per batch. Four AllToAlls (one per half-batch
  of tokens) redistribute from head-sharded to token-sharded; core c then
  projects its 128-token slices with the full Wout.

All matmul operands are bf16 (fp32 PSUM accumulation, ~4e-3 rel err).
The softmax denominator is accumulated on the vector engine (fp16) with
one [128x128] ones-matmul per q-chunk instead of one per k-block.
Host-side pre-swizzled DRAM layouts give contiguous >=1KB DMA lines.
The schedule drains attention as early as the projection allows: q-chunks
0/1 of each batch run inside that batch's projection window, so three of
the four AllToAlls fire before the tail; the tail is batch-1's heavy-half
attention with both batches' output projection as PE filler, and only the
last AllToAll (plus its 15us projection) is exposed.
"""

import numpy as np
import ml_dtypes

import concourse.bass as bass
import concourse.mybir as mybir
import concourse.tile as tile
from concourse import bacc
from concourse.bass_utils import run_bass_kernel_spmd

B, T, D, H, Dh = 2, 2048, 2048, 16, 128
NT = B * T                  # 4096 tokens total
W = 8                       # cores
HL = H // W                 # 2 heads per core
CQKV = 3 * HL * Dh          # 768 qkv columns per core
NCC = CQKV // 128           # 6 column groups of the qkv weight shard
KO = D // 128               # 16 contraction subtiles
QC = 512                    # attention q-chunk
NQC = T // QC               # 4 q-chunks per batch
HT = T // 2                 # half-batch token span (one AllToAll each)
TOKH = HT // W              # 128 tokens per core per half-batch exchange
SCALE = 1.0 / float(np.sqrt(Dh))

# projection token chunks per batch: two 256s first (small startup critical
# path + feeds attention q-chunk 0 early), then three 512s
CHUNKS = [(0, 256), (256, 256), (512, 512), (1024, 512), (1536, 512)]

F32 = mybir.dt.float32
BF16 = mybir.dt.bfloat16
F16 = mybir.dt.float16


def _build():
    nc = bacc.Bacc("TRN2", target_bir_lowering=False, debug=False,
                   enable_asserts=True, num_devices=W)
    # pre-swizzled layouts (see kernel() for the host-side transposes)
    xp = nc.dram_tensor("xp", [128, KO, NT], BF16, kind="ExternalInput").ap()
    wqkv = nc.dram_tensor("wqkv", [128, NCC, KO, 128], BF16, kind="ExternalInput").ap()
    bqkv = nc.dram_tensor("bqkv", [CQKV], F32, kind="ExternalInput").ap()
    wout = nc.dram_tensor("wout", [128, 4, KO, 512], BF16, kind="ExternalInput").ap()
    masktri = nc.dram_tensor("masktri", [128, 128], BF16, kind="ExternalInput").ap()
    ones16 = nc.dram_tensor("ones16", [128, 128], F16, kind="ExternalInput").ap()
    bvbc = nc.dram_tensor("bvbc", [128, HL * Dh], F32, kind="ExternalInput").ap()
    boutbc = nc.dram_tensor("boutbc", [128, D], F32, kind="ExternalInput").ap()
    # rows [(b*2+half)*TOKH ...): tokens [half*HT + c*TOKH ...) of batch b
    out = nc.dram_tensor("out", [B * 2 * TOKH, D], F32, kind="ExternalOutput").ap()

    with tile.TileContext(nc) as tc:
        with tc.tile_pool(name="persist", bufs=1) as persist, \
             tc.tile_pool(name="dram", bufs=1, space="DRAM") as dram_pool:
            mask_sb = persist.tile([128, 128], BF16)
            ones_sb = persist.tile([128, 128], F16)
            bqk_sb = persist.tile([128, 2 * HL], F32)      # Q,K bias (col on partition)
            bv_sb = persist.tile([128, HL * Dh], F32)      # V bias pre-broadcast
            bout_sb = persist.tile([128, D], F32)

            # constants go on the (otherwise idle) gpsimd queue so the sync
            # queue serves the startup-critical weight/x loads immediately
            nc.gpsimd.dma_start(mask_sb[:], masktri)
            nc.gpsimd.dma_start(ones_sb[:], ones16)
            nc.gpsimd.dma_start(bqk_sb[:], bqkv[0:2 * HL * 128].rearrange("(cc p) -> p cc", p=128))
            nc.gpsimd.dma_start(bv_sb[:], bvbc)
            nc.gpsimd.dma_start(bout_sb[:], boutbc)

            a2a_in = [[dram_pool.tile([W, HL * 128, TOKH], BF16, name=f"a2a_in{b}{h}")
                       for h in range(2)] for b in range(B)]
            a2a_out = [[dram_pool.tile([W, HL * 128, TOKH], BF16, name=f"a2a_out{b}{h}")
                        for h in range(2)] for b in range(B)]

            def alloc_qkv(pool):
                qT = pool.tile([128, HL, T], BF16, name="qT")
                kT = pool.tile([128, HL, T], BF16, name="kT")
                v = pool.tile([128, HL, T // 128, Dh], BF16, name="v")
                return qT, kT, v

            def emit_proj_chunk(qkv, wq_sb, x_pool, proj_psum, b, chunk, dma_eng):
                """Project one token chunk of batch b into (qT, kT, v)."""
                qT_sb, kT_sb, v_sb = qkv
                toff, tc_sz = chunk
                t0 = b * T + toff
                x_sb = x_pool.tile([128, KO, tc_sz], BF16, name="x_sb")
                dma_eng.dma_start(x_sb[:], xp[:, :, t0:t0 + tc_sz])
                for cc in range(2 * HL):
                    ps = proj_psum.tile([128, tc_sz], F32, name="proj_ps")
                    for ko in range(KO):
                        nc.tensor.matmul(
                            ps[:], wq_sb[:, cc, ko, :],
                            x_sb[:, ko, :], start=(ko == 0), stop=(ko == KO - 1))
                    dest = qT_sb if cc < HL else kT_sb
                    hl = cc if cc < HL else cc - HL
                    nc.vector.tensor_scalar_add(
                        dest[:, hl, toff:toff + tc_sz], ps[:],
                        bqk_sb[:, cc:cc + 1])
                for tb in range(tc_sz // 128):
                    ps = proj_psum.tile([128, HL * Dh], F32, name="proj_ps")
                    for ko in range(KO):
                        nc.tensor.matmul(
                            ps[:], x_sb[:, ko, tb * 128:(tb + 1) * 128],
                            wq_sb[:, 2 * HL:, ko, :], start=(ko == 0), stop=(ko == KO - 1))
                    idx = toff // 128 + tb
                    nc.vector.tensor_tensor(
                        v_sb[:, :, idx, :],
                        ps[:].rearrange("p (hl d) -> p hl d", hl=HL),
                        bv_sb[:].rearrange("p (hl d) -> p hl d", hl=HL),
                        mybir.AluOpType.add)

            def emit_attn_group(qkv, att_sb, pools, hl, qc):
                """One (head, q-chunk) attention group: S^T -> exp -> P^T V.

                k-blocks are processed in pairs sharing one 2-bank PSUM tile
                so off-diagonal pairs need a single exp over 1024 columns.
                The softmax denominator is accumulated on the vector engine
                (fp16) and broadcast-reduced with one ones-matmul per group.
                """
                qT_sb, kT_sb, v_sb = qkv
                ex_pool, acc_pool, tmp_pool, rden_pool, s_psum, av_psum, den_psum = pools
                q0 = qc * QC
                nkb = (qc + 1) * (QC // 128)
                ps_av = av_psum.tile([128, QC], F32, name="ps_av")
                acc = acc_pool.tile([128, QC], F16, name="acc")
                first_acc = True
                for kbp in range(nkb // 2):
                    kbs = (2 * kbp, 2 * kbp + 1)
                    os_ = [kb - qc * (QC // 128) for kb in kbs]
                    vss = [max(0, o) * 128 for o in os_]
                    ps_s2 = s_psum.tile([128, 2, QC], F32, name="ps_s2")
                    ex2 = ex_pool.tile([128, 2, QC], BF16, name="ex2")
                    for i, kb in enumerate(kbs):
                        nc.tensor.matmul(
                            ps_s2[:, i, vss[i]:], kT_sb[:, hl, kb * 128:(kb + 1) * 128],
                            qT_sb[:, hl, q0 + vss[i]:q0 + QC], start=True, stop=True)
                    if vss[0] == 0 and vss[1] == 0:
                        nc.scalar.activation(
                            ex2[:], ps_s2[:], mybir.ActivationFunctionType.Exp,
                            scale=SCALE)
                    else:
                        for i in range(2):
                            nc.scalar.activation(
                                ex2[:, i, vss[i]:], ps_s2[:, i, vss[i]:],
                                mybir.ActivationFunctionType.Exp, scale=SCALE)
                    for i, kb in enumerate(kbs):
                        if os_[i] >= 0:
                            nc.vector.tensor_tensor(
                                ex2[:, i, vss[i]:vss[i] + 128],
                                ex2[:, i, vss[i]:vss[i] + 128], mask_sb[:],
                                mybir.AluOpType.mult)
                        nc.tensor.matmul(
                            ps_av[:, vss[i]:], v_sb[:, hl, kb, :], ex2[:, i, vss[i]:],
                            start=(kb == 0), stop=(kb == nkb - 1))
                    # denominator partial sums on DVE (fp16)
                    if vss[0] == 0 and vss[1] == 0:
                        if first_acc:
                            nc.vector.tensor_tensor(
                                acc[:], ex2[:, 0, :], ex2[:, 1, :],
                                mybir.AluOpType.add)
                            first_acc = False
                        else:
                            t = tmp_pool.tile([128, QC], F16, name="psum16")
                            nc.vector.tensor_tensor(
                                t[:], ex2[:, 0, :], ex2[:, 1, :],
                                mybir.AluOpType.add)
                            nc.vector.tensor_tensor(
                                acc[:], acc[:], t[:], mybir.AluOpType.add)
                    else:
                        if first_acc:  # qc==0 pair 0: vss == (0, 128)
                            nc.vector.tensor_copy(acc[:], ex2[:, 0, :])
                            nc.vector.tensor_tensor(
                                acc[:, vss[1]:], acc[:, vss[1]:],
                                ex2[:, 1, vss[1]:], mybir.AluOpType.add)
                            first_acc = False
                        else:
                            for i in range(2):
                                nc.vector.tensor_tensor(
                                    acc[:, vss[i]:], acc[:, vss[i]:],
                                    ex2[:, i, vss[i]:], mybir.AluOpType.add)
                ps_den = den_psum.tile([128, QC], F32, name="ps_den")
                nc.tensor.matmul(ps_den[:], ones_sb[:], acc[:], start=True, stop=True)
                rden = rden_pool.tile([128, QC], F32, name="rden")
                nc.vector.reciprocal_approx_fast(rden[:], ps_den[:])
                nc.vector.tensor_tensor(
                    att_sb[:, hl, q0:q0 + QC], ps_av[:], rden[:],
                    mybir.AluOpType.mult)

            def emit_a2a(att_sb, b, half):
                for hl in range(HL):
                    nc.gpsimd.dma_start(
                        a2a_in[b][half][:, hl * 128:(hl + 1) * 128, :].rearrange(
                            "r p t -> p r t"),
                        att_sb[:, hl, half * HT:(half + 1) * HT].rearrange(
                            "p (r t) -> p r t", r=W))
                nc.gpsimd.collective_compute(
                    "AllToAll", mybir.AluOpType.bypass,
                    replica_groups=[list(range(W))],
                    ins=[a2a_in[b][half][:].opt()], outs=[a2a_out[b][half][:].opt()])

            with tc.tile_pool(name="qkv1_pool", bufs=1) as qkv1_pool:
                qkv1 = alloc_qkv(qkv1_pool)
                with tc.tile_pool(name="att1_pool", bufs=1) as att1_pool:
                    att1_sb = att1_pool.tile([128, HL, T], BF16)
                    with tc.tile_pool(name="qkv0_pool", bufs=1) as qkv0_pool:
                        qkv0 = alloc_qkv(qkv0_pool)
                        with tc.tile_pool(name="att0_pool", bufs=1) as att0_pool:
                            att0_sb = att0_pool.tile([128, HL, T], BF16)
                            with tc.tile_pool(name="wq_pool", bufs=1) as wq_pool, \
                                 tc.tile_pool(name="x_pool", bufs=3) as x_pool, \
                                 tc.tile_pool(name="proj_psum", bufs=2, space="PSUM") as proj_psum, \
                                 tc.tile_pool(name="ex0_pool", bufs=3) as ex0_pool, \
                                 tc.tile_pool(name="acc0_pool", bufs=2) as acc0_pool, \
                                 tc.tile_pool(name="tmp0_pool", bufs=2) as tmp0_pool, \
                                 tc.tile_pool(name="rden0_pool", bufs=2) as rden0_pool, \
                                 tc.tile_pool(name="s0_psum", bufs=2, space="PSUM") as s0_psum, \
                                 tc.tile_pool(name="av0_psum", bufs=1, space="PSUM") as av0_psum, \
                                 tc.tile_pool(name="d0_psum", bufs=1, space="PSUM") as d0_psum:
                                wq_sb = wq_pool.tile([128, NCC, KO, 128], BF16, name="wq_sb")
                                # split by column group: psum group cc starts
                                # after only its 0.5MB weight slice arrives
                                for cc in range(NCC):
                                    nc.sync.dma_start(wq_sb[:, cc], wqkv[:, cc])
                                pools0 = (ex0_pool, acc0_pool, tmp0_pool, rden0_pool,
                                          s0_psum, av0_psum, d0_psum)
                                # ---- phase P0: batch-0 projection + its early
                                # attention q-chunks (first x chunks race on the
                                # scalar DMA queue so the PE starts early)
                                emit_proj_chunk(qkv0, wq_sb, x_pool, proj_psum, 0, CHUNKS[0], nc.scalar)
                                emit_proj_chunk(qkv0, wq_sb, x_pool, proj_psum, 0, CHUNKS[1], nc.scalar)
                                emit_attn_group(qkv0, att0_sb, pools0, 0, 0)
                                emit_attn_group(qkv0, att0_sb, pools0, 1, 0)
                                emit_proj_chunk(qkv0, wq_sb, x_pool, proj_psum, 0, CHUNKS[2], nc.sync)
                                emit_attn_group(qkv0, att0_sb, pools0, 0, 1)
                                emit_attn_group(qkv0, att0_sb, pools0, 1, 1)
                                emit_proj_chunk(qkv0, wq_sb, x_pool, proj_psum, 0, CHUNKS[3], nc.sync)
                                emit_proj_chunk(qkv0, wq_sb, x_pool, proj_psum, 0, CHUNKS[4], nc.sync)
                                emit_a2a(att0_sb, 0, 0)
                                # ---- phase P1: batch-1 projection + b0 heavy-half
                                # attention + b1 light-half attention
                                emit_proj_chunk(qkv1, wq_sb, x_pool, proj_psum, 1, CHUNKS[0], nc.sync)
                                emit_proj_chunk(qkv1, wq_sb, x_pool, proj_psum, 1, CHUNKS[1], nc.sync)
                                emit_attn_group(qkv0, att0_sb, pools0, 0, 3)
                                emit_attn_group(qkv0, att0_sb, pools0, 1, 3)
                                emit_proj_chunk(qkv1, wq_sb, x_pool, proj_psum, 1, CHUNKS[2], nc.sync)
                                emit_attn_group(qkv0, att0_sb, pools0, 0, 2)
                                emit_attn_group(qkv0, att0_sb, pools0, 1, 2)
                                emit_a2a(att0_sb, 0, 1)
                                emit_attn_group(qkv1, att1_sb, pools0, 0, 0)
                                emit_attn_group(qkv1, att1_sb, pools0, 1, 0)
                                emit_proj_chunk(qkv1, wq_sb, x_pool, proj_psum, 1, CHUNKS[3], nc.sync)
                                emit_attn_group(qkv1, att1_sb, pools0, 0, 1)
                                emit_attn_group(qkv1, att1_sb, pools0, 1, 1)
                                emit_proj_chunk(qkv1, wq_sb, x_pool, proj_psum, 1, CHUNKS[4], nc.sync)
                                emit_a2a(att1_sb, 1, 0)
                        # qkv0/att0 freed; tail: b1 heavy-half attention with
                        # both batches' output projection as PE filler
                    with tc.tile_pool(name="wout_pool", bufs=1) as wout_pool, \
                         tc.tile_pool(name="attall_pool", bufs=4) as attall_pool, \
                         tc.tile_pool(name="o_pool", bufs=3) as o_pool, \
                         tc.tile_pool(name="out_psum", bufs=2, space="PSUM") as out_psum, \
                         tc.tile_pool(name="ex1_pool", bufs=3) as ex1_pool, \
                         tc.tile_pool(name="acc1_pool", bufs=2) as acc1_pool, \
                         tc.tile_pool(name="tmp1_pool", bufs=2) as tmp1_pool, \
                         tc.tile_pool(name="rden1_pool", bufs=2) as rden1_pool, \
                         tc.tile_pool(name="s1_psum", bufs=2, space="PSUM") as s1_psum, \
                         tc.tile_pool(name="av1_psum", bufs=1, space="PSUM") as av1_psum, \
                         tc.tile_pool(name="d1_psum", bufs=1, space="PSUM") as d1_psum:
                        wout_sb = wout_pool.tile([128, 4, KO, 512], BF16, name="wout_sb")
                        for colc in range(4):
                            nc.sync.dma_start(wout_sb[:, colc], wout[:, colc])
                        pools1 = (ex1_pool, acc1_pool, tmp1_pool, rden1_pool,
                                  s1_psum, av1_psum, d1_psum)

                        attall = {}

                        def load_attall(b, half):
                            t = attall_pool.tile([128, KO, TOKH], BF16, name="attall")
                            nc.sync.dma_start(
                                t[:],
                                a2a_out[b][half][:].rearrange(
                                    "r (x p) t -> p (r x) t", x=HL, p=128))
                            attall[(b, half)] = t

                        def emit_outproj_group(b, half, colc):
                            ps_o = out_psum.tile([128, 512], F32, name="ps_o")
                            for ko in range(KO):
                                nc.tensor.matmul(
                                    ps_o[:], attall[(b, half)][:, ko, :],
                                    wout_sb[:, colc, ko, :],
                                    start=(ko == 0), stop=(ko == KO - 1))
                            o_sb = o_pool.tile([128, 512], F32, name="o_sb")
                            nc.vector.tensor_tensor(
                                o_sb[:], ps_o[:],
                                bout_sb[:, colc * 512:(colc + 1) * 512],
                                mybir.AluOpType.add)
                            nc.sync.dma_start(
                                out[(b * 2 + half) * TOKH:(b * 2 + half + 1) * TOKH,
                                    colc * 512:(colc + 1) * 512],
                                o_sb[:])

                        load_attall(0, 0)   # A2A fired at end of P0
                        load_attall(0, 1)   # fired mid-P1
                        load_attall(1, 0)   # fired at end of P1
                        # b1 heavy-half attention first (highest priority) so
                        # the last A2A fires ASAP; outproj groups are emitted
                        # after and fill the exp gaps + the A2A window
                        emit_attn_group(qkv1, att1_sb, pools1, 0, 2)
                        emit_attn_group(qkv1, att1_sb, pools1, 1, 2)
                        emit_attn_group(qkv1, att1_sb, pools1, 0, 3)
                        emit_attn_group(qkv1, att1_sb, pools1, 1, 3)
                        emit_a2a(att1_sb, 1, 1)
                        load_attall(1, 1)
                        for colc in range(4):
                            emit_outproj_group(0, 0, colc)
                        for colc in range(4):
                            emit_outproj_group(0, 1, colc)
                        for colc in range(4):
                            emit_outproj_group(1, 0, colc)
                        for colc in range(4):
                            emit_outproj_group(1, 1, colc)
    nc.compile()
    return nc


_CACHED_NC = None


def kernel(x, Wqkv, bqkv, Wout, bout):
    global _CACHED_NC
    x = np.asarray(x, dtype=np.float32)
    Wqkv = np.asarray(Wqkv, dtype=np.float32)
    bqkv = np.asarray(bqkv, dtype=np.float32)
    Wout = np.asarray(Wout, dtype=np.float32)
    bout = np.asarray(bout, dtype=np.float32)

    if _CACHED_NC is None:
        _CACHED_NC = _build()
    nc = _CACHED_NC

    bf16 = ml_dtypes.bfloat16
    # xp[p, ko, t] = x[t, ko*128+p]
    xp = np.ascontiguousarray(
        x.reshape(NT, KO, 128).transpose(2, 1, 0)).astype(bf16)
    wq4 = Wqkv.reshape(D, 3, H, Dh)                              # col = which, head, dh
    bq4 = bqkv.reshape(3, H, Dh)
    # wout_pre[p, colc, ko, c] = Wout[ko*128+p, colc*512+c]
    wout_pre = np.ascontiguousarray(
        Wout.reshape(KO, 128, 4, 512).transpose(1, 2, 0, 3)).astype(bf16)
    kl = np.arange(128)[:, None]
    jl = np.arange(128)[None, :]
    masktri = (jl >= kl).astype(bf16)

    in_maps = []
    for c in range(W):
        wshard = np.ascontiguousarray(
            wq4[:, :, HL * c:HL * c + HL, :].reshape(D, CQKV))
        # wpre[p, cc, ko, ccol] = wshard[ko*128+p, cc*128+ccol]
        wpre = np.ascontiguousarray(
            wshard.reshape(KO, 128, NCC, 128).transpose(1, 2, 0, 3)).astype(bf16)
        bshard = np.ascontiguousarray(
            bq4[:, HL * c:HL * c + HL, :].reshape(CQKV))
        in_maps.append({
            "xp": xp, "wqkv": wpre, "bqkv": bshard,
            "wout": wout_pre, "masktri": masktri,
            "ones16": np.ones((128, 128), np.float16),
            "bvbc": np.tile(bshard[2 * HL * 128:][None, :], (128, 1)),
            "boutbc": np.tile(bout[None, :], (128, 1)),
        })

    res = run_bass_kernel_spmd(nc, in_maps, core_ids=list(range(W)))
    # res[c]["out"] rows [(b*2+h)*TOKH ...) = tokens [h*HT + c*TOKH ...) of batch b
    full = np.empty((B, T, D), np.float32)
    for c in range(W):
        for b in range(B):
            for h in range(2):
                full[b, h * HT + c * TOKH:h * HT + (c + 1) * TOKH] = \
                    res.results[c]["out"][(b * 2 + h) * TOKH:(b * 2 + h + 1) * TOKH]
    return full


# revision 11
# speedup vs baseline: 1.4733x; 1.0074x over previous
"""Causal self-attention kernel for 8 Trainium2 NeuronCores.

Problem: B=2, T=2048, D=2048, H=16, Dh=128, fp32 in/out.
  qkv = x @ Wqkv + bqkv ; per-head causal attention ; out = att @ Wout + bout

Sharding (tensor parallel over heads + AllToAll before out_proj):
  Core c owns heads {2c, 2c+1}. Each core computes, for all 4096 tokens,
  Q^T/K^T (head-dim on partitions) and V (token-dim on partitions) for its
  two heads via the QKV projection with its 768-column shard of Wqkv, runs
  causal attention locally (scores computed transposed: S^T[k,q]), and
  produces att^T [256, 2048] per batch. Four AllToAlls (one per half-batch
  of tokens) redistribute from head-sharded to token-sharded; core c then
  projects its 128-token slices with the full Wout.

All matmul operands are bf16 (fp32 PSUM accumulation, ~4e-3 rel err).
The softmax denominator is accumulated on the vector engine (fp16) with
one [128x128] ones-matmul per q-chunk instead of one per k-block.
Host-side pre-swizzled DRAM layouts give contiguous >=1KB DMA lines.
The schedule drains attention as early as the projection allows: q-chunks
0/1 of each batch run inside that batch's projection window, so three of
the four AllToAlls fire before the tail; the tail is batch-1's heavy-half
attention with both batches' output projection as PE filler, and only the
last AllToAll (plus its 15us projection) is exposed.
"""

import numpy as np
import ml_dtypes

import concourse.bass as bass
import concourse.mybir as mybir
import concourse.tile as tile
from concourse import bacc
from concourse.bass_utils import run_bass_kernel_spmd

B, T, D, H, Dh = 2, 2048, 2048, 16, 128
NT = B * T                  # 4096 tokens total
W = 8                       # cores
HL = H // W                 # 2 heads per core
CQKV = 3 * HL * Dh          # 768 qkv columns per core
NCC = CQKV // 128           # 6 column groups of the qkv weight shard
KO = D // 128               # 16 contraction subtiles
QC = 512                    # attention q-chunk
NQC = T // QC               # 4 q-chunks per batch
HT = T // 2                 # half-batch token span (one AllToAll each)
TOKH = HT // W              # 128 tokens per core per half-batch exchange
SCALE = 1.0 / float(np.sqrt(Dh))

# projection token chunks per batch: two 256s first (small startup critical
# path + feeds attention q-chunk 0 early), then three 512s
CHUNKS = [(0, 256), (256, 256), (512, 512), (1024, 512), (1536, 512)]

F32 = mybir.dt.float32
BF16 = mybir.dt.bfloat16
F16 = mybir.dt.float16


def _build():
    nc = bacc.Bacc("TRN2", target_bir_lowering=False, debug=False,
                   enable_asserts=True, num_devices=W)
    # pre-swizzled layouts (see kernel() for the host-side transposes)
    xp = nc.dram_tensor("xp", [128, KO, NT], BF16, kind="ExternalInput").ap()
    wqkv = nc.dram_tensor("wqkv", [128, NCC, KO, 128], BF16, kind="ExternalInput").ap()
    bqkv = nc.dram_tensor("bqkv", [CQKV], F32, kind="ExternalInput").ap()
    wout = nc.dram_tensor("wout", [128, 4, KO, 512], BF16, kind="ExternalInput").ap()
    masktri = nc.dram_tensor("masktri", [128, 128], BF16, kind="ExternalInput").ap()
    ones16 = nc.dram_tensor("ones16", [128, 128], F16, kind="ExternalInput").ap()
    bvbc = nc.dram_tensor("bvbc", [128, HL * Dh], F32, kind="ExternalInput").ap()
    boutbc = nc.dram_tensor("boutbc", [128, D], F32, kind="ExternalInput").ap()
    # rows [(b*2+half)*TOKH ...): tokens [half*HT + c*TOKH ...) of batch b
    out = nc.dram_tensor("out", [B * 2 * TOKH, D], F32, kind="ExternalOutput").ap()

    with tile.TileContext(nc) as tc:
        with tc.tile_pool(name="persist", bufs=1) as persist, \
             tc.tile_pool(name="dram", bufs=1, space="DRAM") as dram_pool:
            mask_sb = persist.tile([128, 128], BF16)
            ones_sb = persist.tile([128, 128], F16)
            bqk_sb = persist.tile([128, 2 * HL], F32)      # Q,K bias (col on partition)
            bv_sb = persist.tile([128, HL * Dh], F32)      # V bias pre-broadcast
            bout_sb = persist.tile([128, D], F32)

            # constants go on the (otherwise idle) gpsimd queue so the sync
            # queue serves the startup-critical weight/x loads immediately
            nc.gpsimd.dma_start(mask_sb[:], masktri)
            nc.gpsimd.dma_start(ones_sb[:], ones16)
            nc.gpsimd.dma_start(bqk_sb[:], bqkv[0:2 * HL * 128].rearrange("(cc p) -> p cc", p=128))
            nc.gpsimd.dma_start(bv_sb[:], bvbc)
            # boutbc (1MB) is only needed by the tail out-projection; loaded there

            a2a_in = [[dram_pool.tile([W, HL * 128, TOKH], BF16, name=f"a2a_in{b}{h}")
                       for h in range(2)] for b in range(B)]
            a2a_out = [[dram_pool.tile([W, HL * 128, TOKH], BF16, name=f"a2a_out{b}{h}")
                        for h in range(2)] for b in range(B)]

            def alloc_qkv(pool):
                qT = pool.tile([128, HL, T], BF16, name="qT")
                kT = pool.tile([128, HL, T], BF16, name="kT")
                v = pool.tile([128, HL, T // 128, Dh], BF16, name="v")
                return qT, kT, v

            def emit_proj_chunk(qkv, wq_sb, x_pool, proj_psum, b, chunk, dma_eng):
                """Project one token chunk of batch b into (qT, kT, v)."""
                qT_sb, kT_sb, v_sb = qkv
                toff, tc_sz = chunk
                t0 = b * T + toff
                x_sb = x_pool.tile([128, KO, tc_sz], BF16, name="x_sb")
                if toff < 512 and b == 0:
                    # startup: split along ko so the first PSUM group (which
                    # consumes ko slices in order) starts after ~0.25MB
                    for k4 in range(0, KO, 4):
                        dma_eng.dma_start(x_sb[:, k4:k4 + 4, :],
                                          xp[:, k4:k4 + 4, t0:t0 + tc_sz])
                else:
                    dma_eng.dma_start(x_sb[:], xp[:, :, t0:t0 + tc_sz])
                for cc in range(2 * HL):
                    ps = proj_psum.tile([128, tc_sz], F32, name="proj_ps")
                    for ko in range(KO):
                        nc.tensor.matmul(
                            ps[:], wq_sb[:, cc, ko, :],
                            x_sb[:, ko, :], start=(ko == 0), stop=(ko == KO - 1))
                    dest = qT_sb if cc < HL else kT_sb
                    hl = cc if cc < HL else cc - HL
                    nc.vector.tensor_scalar_add(
                        dest[:, hl, toff:toff + tc_sz], ps[:],
                        bqk_sb[:, cc:cc + 1])
                for tb in range(tc_sz // 128):
                    ps = proj_psum.tile([128, HL * Dh], F32, name="proj_ps")
                    for ko in range(KO):
                        nc.tensor.matmul(
                            ps[:], x_sb[:, ko, tb * 128:(tb + 1) * 128],
                            wq_sb[:, 2 * HL:, ko, :], start=(ko == 0), stop=(ko == KO - 1))
                    idx = toff // 128 + tb
                    nc.vector.tensor_tensor(
                        v_sb[:, :, idx, :],
                        ps[:].rearrange("p (hl d) -> p hl d", hl=HL),
                        bv_sb[:].rearrange("p (hl d) -> p hl d", hl=HL),
                        mybir.AluOpType.add)

            def emit_attn_group(qkv, att_sb, pools, hl, qc):
                """One (head, q-chunk) attention group: S^T -> exp -> P^T V.

                k-blocks are processed in pairs sharing one 2-bank PSUM tile
                so off-diagonal pairs need a single exp over 1024 columns.
                The softmax denominator is accumulated on the vector engine
                (fp16) and broadcast-reduced with one ones-matmul per group.
                """
                qT_sb, kT_sb, v_sb = qkv
                ex_pool, acc_pool, tmp_pool, rden_pool, s_psum, av_psum, den_psum = pools
                q0 = qc * QC
                nkb = (qc + 1) * (QC // 128)
                ps_av = av_psum.tile([128, QC], F32, name="ps_av")
                acc = acc_pool.tile([128, QC], F16, name="acc")
                first_acc = True
                for kbp in range(nkb // 2):
                    kbs = (2 * kbp, 2 * kbp + 1)
                    os_ = [kb - qc * (QC // 128) for kb in kbs]
                    vss = [max(0, o) * 128 for o in os_]
                    ps_s2 = s_psum.tile([128, 2, QC], F32, name="ps_s2")
                    ex2 = ex_pool.tile([128, 2, QC], BF16, name="ex2")
                    for i, kb in enumerate(kbs):
                        nc.tensor.matmul(
                            ps_s2[:, i, vss[i]:], kT_sb[:, hl, kb * 128:(kb + 1) * 128],
                            qT_sb[:, hl, q0 + vss[i]:q0 + QC], start=True, stop=True)
                    if vss[0] == 0 and vss[1] == 0:
                        nc.scalar.activation(
                            ex2[:], ps_s2[:], mybir.ActivationFunctionType.Exp,
                            scale=SCALE)
                    else:
                        for i in range(2):
                            nc.scalar.activation(
                                ex2[:, i, vss[i]:], ps_s2[:, i, vss[i]:],
                                mybir.ActivationFunctionType.Exp, scale=SCALE)
                    for i, kb in enumerate(kbs):
                        if os_[i] >= 0:
                            nc.vector.tensor_tensor(
                                ex2[:, i, vss[i]:vss[i] + 128],
                                ex2[:, i, vss[i]:vss[i] + 128], mask_sb[:],
                                mybir.AluOpType.mult)
                        nc.tensor.matmul(
                            ps_av[:, vss[i]:], v_sb[:, hl, kb, :], ex2[:, i, vss[i]:],
                            start=(kb == 0), stop=(kb == nkb - 1))
                    # denominator partial sums on DVE (fp16)
                    if vss[0] == 0 and vss[1] == 0:
                        if first_acc:
                            nc.vector.tensor_tensor(
                                acc[:], ex2[:, 0, :], ex2[:, 1, :],
                                mybir.AluOpType.add)
                            first_acc = False
                        else:
                            t = tmp_pool.tile([128, QC], F16, name="psum16")
                            nc.vector.tensor_tensor(
                                t[:], ex2[:, 0, :], ex2[:, 1, :],
                                mybir.AluOpType.add)
                            nc.vector.tensor_tensor(
                                acc[:], acc[:], t[:], mybir.AluOpType.add)
                    else:
                        if first_acc:  # qc==0 pair 0: vss == (0, 128)
                            nc.vector.tensor_copy(acc[:], ex2[:, 0, :])
                            nc.vector.tensor_tensor(
                                acc[:, vss[1]:], acc[:, vss[1]:],
                                ex2[:, 1, vss[1]:], mybir.AluOpType.add)
                            first_acc = False
                        else:
                            for i in range(2):
                                nc.vector.tensor_tensor(
                                    acc[:, vss[i]:], acc[:, vss[i]:],
                                    ex2[:, i, vss[i]:], mybir.AluOpType.add)
                ps_den = den_psum.tile([128, QC], F32, name="ps_den")
                nc.tensor.matmul(ps_den[:], ones_sb[:], acc[:], start=True, stop=True)
                rden = rden_pool.tile([128, QC], F32, name="rden")
                nc.vector.reciprocal_approx_fast(rden[:], ps_den[:])
                nc.vector.tensor_tensor(
                    att_sb[:, hl, q0:q0 + QC], ps_av[:], rden[:],
                    mybir.AluOpType.mult)

            def emit_a2a(att_sb, b, half):
                for hl in range(HL):
                    nc.gpsimd.dma_start(
                        a2a_in[b][half][:, hl * 128:(hl + 1) * 128, :].rearrange(
                            "r p t -> p r t"),
                        att_sb[:, hl, half * HT:(half + 1) * HT].rearrange(
                            "p (r t) -> p r t", r=W))
                nc.gpsimd.collective_compute(
                    "AllToAll", mybir.AluOpType.bypass,
                    replica_groups=[list(range(W))],
                    ins=[a2a_in[b][half][:].opt()], outs=[a2a_out[b][half][:].opt()])

            with tc.tile_pool(name="qkv1_pool", bufs=1) as qkv1_pool:
                qkv1 = alloc_qkv(qkv1_pool)
                with tc.tile_pool(name="att1_pool", bufs=1) as att1_pool:
                    att1_sb = att1_pool.tile([128, HL, T], BF16)
                    with tc.tile_pool(name="qkv0_pool", bufs=1) as qkv0_pool:
                        qkv0 = alloc_qkv(qkv0_pool)
                        with tc.tile_pool(name="att0_pool", bufs=1) as att0_pool:
                            att0_sb = att0_pool.tile([128, HL, T], BF16)
                            with tc.tile_pool(name="wq_pool", bufs=1) as wq_pool, \
                                 tc.tile_pool(name="x_pool", bufs=3) as x_pool, \
                                 tc.tile_pool(name="proj_psum", bufs=2, space="PSUM") as proj_psum, \
                                 tc.tile_pool(name="ex0_pool", bufs=3) as ex0_pool, \
                                 tc.tile_pool(name="acc0_pool", bufs=2) as acc0_pool, \
                                 tc.tile_pool(name="tmp0_pool", bufs=2) as tmp0_pool, \
                                 tc.tile_pool(name="rden0_pool", bufs=2) as rden0_pool, \
                                 tc.tile_pool(name="s0_psum", bufs=2, space="PSUM") as s0_psum, \
                                 tc.tile_pool(name="av0_psum", bufs=1, space="PSUM") as av0_psum, \
                                 tc.tile_pool(name="d0_psum", bufs=1, space="PSUM") as d0_psum:
                                wq_sb = wq_pool.tile([128, NCC, KO, 128], BF16, name="wq_sb")
                                # split by column group: psum group cc starts
                                # after only its 0.5MB weight slice arrives
                                for cc in range(NCC):
                                    nc.sync.dma_start(wq_sb[:, cc], wqkv[:, cc])
                                pools0 = (ex0_pool, acc0_pool, tmp0_pool, rden0_pool,
                                          s0_psum, av0_psum, d0_psum)
                                # ---- phase P0: batch-0 projection + its early
                                # attention q-chunks (first x chunks race on the
                                # scalar DMA queue so the PE starts early)
                                emit_proj_chunk(qkv0, wq_sb, x_pool, proj_psum, 0, CHUNKS[0], nc.scalar)
                                emit_proj_chunk(qkv0, wq_sb, x_pool, proj_psum, 0, CHUNKS[1], nc.scalar)
                                emit_attn_group(qkv0, att0_sb, pools0, 0, 0)
                                emit_attn_group(qkv0, att0_sb, pools0, 1, 0)
                                emit_proj_chunk(qkv0, wq_sb, x_pool, proj_psum, 0, CHUNKS[2], nc.sync)
                                emit_attn_group(qkv0, att0_sb, pools0, 0, 1)
                                emit_attn_group(qkv0, att0_sb, pools0, 1, 1)
                                emit_proj_chunk(qkv0, wq_sb, x_pool, proj_psum, 0, CHUNKS[3], nc.sync)
                                emit_proj_chunk(qkv0, wq_sb, x_pool, proj_psum, 0, CHUNKS[4], nc.sync)
                                emit_a2a(att0_sb, 0, 0)
                                # ---- phase P1: batch-1 projection + b0 heavy-half
                                # attention + b1 light-half attention
                                emit_proj_chunk(qkv1, wq_sb, x_pool, proj_psum, 1, CHUNKS[0], nc.sync)
                                emit_proj_chunk(qkv1, wq_sb, x_pool, proj_psum, 1, CHUNKS[1], nc.sync)
                                emit_attn_group(qkv0, att0_sb, pools0, 0, 3)
                                emit_attn_group(qkv0, att0_sb, pools0, 1, 3)
                                emit_proj_chunk(qkv1, wq_sb, x_pool, proj_psum, 1, CHUNKS[2], nc.sync)
                                emit_attn_group(qkv0, att0_sb, pools0, 0, 2)
                                emit_attn_group(qkv0, att0_sb, pools0, 1, 2)
                                emit_a2a(att0_sb, 0, 1)
                                emit_attn_group(qkv1, att1_sb, pools0, 0, 0)
                                emit_attn_group(qkv1, att1_sb, pools0, 1, 0)
                                emit_proj_chunk(qkv1, wq_sb, x_pool, proj_psum, 1, CHUNKS[3], nc.sync)
                                emit_attn_group(qkv1, att1_sb, pools0, 0, 1)
                                emit_attn_group(qkv1, att1_sb, pools0, 1, 1)
                                emit_proj_chunk(qkv1, wq_sb, x_pool, proj_psum, 1, CHUNKS[4], nc.sync)
                                emit_a2a(att1_sb, 1, 0)
                        # qkv0/att0 freed; tail: b1 heavy-half attention with
                        # both batches' output projection as PE filler
                    with tc.tile_pool(name="wout_pool", bufs=1) as wout_pool, \
                         tc.tile_pool(name="attall_pool", bufs=4) as attall_pool, \
                         tc.tile_pool(name="o_pool", bufs=3) as o_pool, \
                         tc.tile_pool(name="out_psum", bufs=2, space="PSUM") as out_psum, \
                         tc.tile_pool(name="ex1_pool", bufs=3) as ex1_pool, \
                         tc.tile_pool(name="acc1_pool", bufs=2) as acc1_pool, \
                         tc.tile_pool(name="tmp1_pool", bufs=2) as tmp1_pool, \
                         tc.tile_pool(name="rden1_pool", bufs=2) as rden1_pool, \
                         tc.tile_pool(name="s1_psum", bufs=2, space="PSUM") as s1_psum, \
                         tc.tile_pool(name="av1_psum", bufs=1, space="PSUM") as av1_psum, \
                         tc.tile_pool(name="d1_psum", bufs=1, space="PSUM") as d1_psum:
                        nc.scalar.dma_start(bout_sb[:], boutbc)
                        wout_sb = wout_pool.tile([128, 4, KO, 512], BF16, name="wout_sb")
                        for colc in range(4):
                            nc.sync.dma_start(wout_sb[:, colc], wout[:, colc])
                        pools1 = (ex1_pool, acc1_pool, tmp1_pool, rden1_pool,
                                  s1_psum, av1_psum, d1_psum)

                        attall = {}

                        def load_attall(b, half):
                            t = attall_pool.tile([128, KO, TOKH], BF16, name="attall")
                            nc.sync.dma_start(
                                t[:],
                                a2a_out[b][half][:].rearrange(
                                    "r (x p) t -> p (r x) t", x=HL, p=128))
                            attall[(b, half)] = t

                        def emit_outproj_group(b, half, colc):
                            ps_o = out_psum.tile([128, 512], F32, name="ps_o")
                            for ko in range(KO):
                                nc.tensor.matmul(
                                    ps_o[:], attall[(b, half)][:, ko, :],
                                    wout_sb[:, colc, ko, :],
                                    start=(ko == 0), stop=(ko == KO - 1))
                            o_sb = o_pool.tile([128, 512], F32, name="o_sb")
                            nc.vector.tensor_tensor(
                                o_sb[:], ps_o[:],
                                bout_sb[:, colc * 512:(colc + 1) * 512],
                                mybir.AluOpType.add)
                            nc.sync.dma_start(
                                out[(b * 2 + half) * TOKH:(b * 2 + half + 1) * TOKH,
                                    colc * 512:(colc + 1) * 512],
                                o_sb[:])

                        load_attall(0, 0)   # A2A fired at end of P0
                        load_attall(0, 1)   # fired mid-P1
                        load_attall(1, 0)   # fired at end of P1
                        # b1 heavy-half attention first (highest priority) so
                        # the last A2A fires ASAP; outproj groups are emitted
                        # after and fill the exp gaps + the A2A window
                        emit_attn_group(qkv1, att1_sb, pools1, 0, 2)
                        emit_attn_group(qkv1, att1_sb, pools1, 1, 2)
                        emit_attn_group(qkv1, att1_sb, pools1, 0, 3)
                        emit_attn_group(qkv1, att1_sb, pools1, 1, 3)
                        emit_a2a(att1_sb, 1, 1)
                        load_attall(1, 1)
                        for colc in range(4):
                            emit_outproj_group(0, 0, colc)
                        for colc in range(4):
                            emit_outproj_group(0, 1, colc)
                        for colc in range(4):
                            emit_outproj_group(1, 0, colc)
                        for colc in range(4):
                            emit_outproj_group(1, 1, colc)
    nc.compile()
    return nc


_CACHED_NC = None


def kernel(x, Wqkv, bqkv, Wout, bout):
    global _CACHED_NC
    x = np.asarray(x, dtype=np.float32)
    Wqkv = np.asarray(Wqkv, dtype=np.float32)
    bqkv = np.asarray(bqkv, dtype=np.float32)
    Wout = np.asarray(Wout, dtype=np.float32)
    bout = np.asarray(bout, dtype=np.float32)

    if _CACHED_NC is None:
        _CACHED_NC = _build()
    nc = _CACHED_NC

    bf16 = ml_dtypes.bfloat16
    # xp[p, ko, t] = x[t, ko*128+p]
    xp = np.ascontiguousarray(
        x.reshape(NT, KO, 128).transpose(2, 1, 0)).astype(bf16)
    wq4 = Wqkv.reshape(D, 3, H, Dh)                              # col = which, head, dh
    bq4 = bqkv.reshape(3, H, Dh)
    # wout_pre[p, colc, ko, c] = Wout[ko*128+p, colc*512+c]
    wout_pre = np.ascontiguousarray(
        Wout.reshape(KO, 128, 4, 512).transpose(1, 2, 0, 3)).astype(bf16)
    kl = np.arange(128)[:, None]
    jl = np.arange(128)[None, :]
    masktri = (jl >= kl).astype(bf16)

    in_maps = []
    for c in range(W):
        wshard = np.ascontiguousarray(
            wq4[:, :, HL * c:HL * c + HL, :].reshape(D, CQKV))
        # wpre[p, cc, ko, ccol] = wshard[ko*128+p, cc*128+ccol]
        wpre = np.ascontiguousarray(
            wshard.reshape(KO, 128, NCC, 128).transpose(1, 2, 0, 3)).astype(bf16)
        bshard = np.ascontiguousarray(
            bq4[:, HL * c:HL * c + HL, :].reshape(CQKV))
        in_maps.append({
            "xp": xp, "wqkv": wpre, "bqkv": bshard,
            "wout": wout_pre, "masktri": masktri,
            "ones16": np.ones((128, 128), np.float16),
            "bvbc": np.tile(bshard[2 * HL * 128:][None, :], (128, 1)),
            "boutbc": np.tile(bout[None, :], (128, 1)),
        })

    res = run_bass_kernel_spmd(nc, in_maps, core_ids=list(range(W)))
    # res[c]["out"] rows [(b*2+h)*TOKH ...) = tokens [h*HT + c*TOKH ...) of batch b
    full = np.empty((B, T, D), np.float32)
    for c in range(W):
        for b in range(B):
            for h in range(2):
                full[b, h * HT + c * TOKH:h * HT + (c + 1) * TOKH] = \
                    res.results[c]["out"][(b * 2 + h) * TOKH:(b * 2 + h + 1) * TOKH]
    return full
